# revision 10
# baseline (speedup 1.0000x reference)
"""Trainium2 Bass kernel for ClinicalStateFormationOperator (v2, pipelined).

Full-input contract: kernel(**inputs) takes the complete (unsharded) numpy
inputs and returns the full [B, T, V, D] output. Work is sharded across 8
NeuronCores as (batch, head-group): core c handles batch c//2 and heads
(c%2)*4 .. (c%2)*4+3. Each core computes its 4 heads' attention and the
partial output projection; the host sums the two partial projections per
batch and adds the output bias.

Math identical to v1 (scores computed transposed with a packed K=128
contraction: content 64 | obs 16 | variable-bias one-hots 32 | time-bias
basis 16; E^T = exp(scores^T); [out^T; denom] = [v_h | ones]^T @ E^T;
divide; partial out-projection). 1/sqrt(hd) and 1/sqrt(od) are folded into
Wq/Woq host-side so all PSUM->SBUF moves are plain copies.

Scheduling (the point of v2): the score->exp->AV chain is software-
pipelined one group ahead (PE emits scores for group i+1 before the AV of
group i), the Activation engine runs the exps plus the early-phase copies
that fit its pre-steady-state idle windows, all other PSUM->SBUF copies
run on DVE (gpsimd cannot access PSUM), and the projection matmuls are
emitted as deadline-tagged fillers inside the attention loop. Per-engine
execution order is pinned to emission order with nosync dependency chains
(see _chain_cb) because the Tile scheduler's region tracker only gives the
first-emitted reader of a region an edge to its writer.

v is stored interleaved [128, KC, head, {v|ones}, 64] so each head's AV
stationary operand [v_h | ones] is one contiguous 128-wide block and each
key chunk needs only a single strided copy from the projection PSUM.

PSUM: 4 banks score double-buffer ([128,2,512] x2) + 2 banks attention-out
([128,512] x2) + 2 banks projections/y ([128,512] x2).
"""

import ml_dtypes
import numpy as np

import concourse.bass as bass
import concourse.mybir as mybir
import concourse.tile as tile
from concourse.bass_utils import run_bass_kernel_spmd
from concourse.instruction_name_ordered_set import InstructionNameOrderedSet

V = 32
T = 48
D = 512
H = 8
HD = D // H          # 64
OD = 16
B = 4
N = T * V            # 1536
HPC = 4              # heads per core
NCORES = 8
SCALE = 1.0 / np.sqrt(HD)
OBS_SCALE = 1.0 / np.sqrt(OD)

F32 = mybir.dt.float32
F32R = mybir.dt.float32r
BF16 = mybir.dt.bfloat16
EXP = mybir.ActivationFunctionType.Exp

# bf16 q/k/v projection inputs: halves the startup DMA (which gates the
# first ~25 groups) for ~1e-2 worst-case relative error, well under the
# 2e-2 gate. Score packs, biases and the out-projection stay float32r.
PROJ_BF16 = True
PDT = BF16 if PROJ_BF16 else F32R

KC = N // 128        # 12 key chunks of 128
QC = N // 512        # 3 query chunks of 512
GPS = 6              # score/exp/AV groups per (head, qchunk): 2 key chunks

# slot order: j outer, h inner (v1 order). All slots need all keys, so the
# hard-early deliveries are fhT + k-proj(m0) only; q/obs/apack columns for
# j1/j2 and the m=1 packs have 12+ groups of slack.
SLOTS = [(j, h) for j in range(QC) for h in range(HPC)]


def _split_waits(nc, max_waits=1):
    """Walrus in this container allows only one sync-wait slot per
    instruction; spill extra waits onto preceding same-engine NoOps."""
    def fix_bb(bb):
        changed = False
        new = []
        for inst in bb.instructions:
            si = inst.sync_info
            if si is not None and len(si.on_wait) > max_waits:
                waits = list(si.on_wait)
                for w in waits[:-max_waits]:
                    new.append(mybir.InstNoOp(
                        name=nc.get_next_instruction_name(),
                        engine=inst.engine, ins=[], outs=[],
                        sync_info=mybir.SyncInfo(on_wait=[w], on_update=[])))
                    changed = True
                si.on_wait = waits[-max_waits:]
            new.append(inst)
        if changed:
            bb.instructions = new
        for sub in getattr(bb, 'blocks', []) or []:
            fix_bb(sub)
    for f in nc.m.functions:
        for bb in f.blocks:
            fix_bb(bb)


def _build(with_bias=False):
    nc = bass.Bass()

    # The Tile scheduler reorders same-engine instructions while its region
    # tracker gives only the FIRST-emitted reader of a region an edge to the
    # writer; a later reader hoisted above that first reader can then run
    # before the data lands (observed: score matmuls reading uninitialized
    # pack rows). Chain every compute instruction to its same-engine
    # predecessor so the per-engine order is exactly emission order, which
    # restores the transitivity the tracker's dedup assumes.
    _chain_last = {}
    _chain_types = (mybir.InstMatmult, mybir.InstTensorCopy,
                    mybir.InstActivation, mybir.InstReciprocal,
                    mybir.InstTensorTensor)
    _chain_engines = (mybir.EngineType.PE, mybir.EngineType.DVE,
                      mybir.EngineType.Activation)
    def _chain_cb(inst):
        if inst.engine in _chain_engines and isinstance(inst, _chain_types):
            prev = _chain_last.get(inst.engine)
            if prev is not None:
                deps = InstructionNameOrderedSet()
                deps.add(prev.name)
                # nosync: same-engine ordering is free (in-order engines);
                # a sync dep here would cost a redundant semaphore wait
                inst.add_nosync_dependencies_from(deps)
            _chain_last[inst.engine] = inst
    nc._state.push_inst_callback(_chain_cb)

    # ---- per-core DRAM I/O (data differs per core, program is SPMD) ----
    fhT = nc.dram_tensor('fhT', [D, N], PDT, kind='ExternalInput')
    foT = nc.dram_tensor('foT', [2, N], F32R, kind='ExternalInput')
    wq = nc.dram_tensor('wq', [D, HPC * HD], PDT, kind='ExternalInput')
    wk = nc.dram_tensor('wk', [D, HPC * HD], PDT, kind='ExternalInput')
    wv = nc.dram_tensor('wv', [D, HPC * HD], PDT, kind='ExternalInput')
    woq = nc.dram_tensor('woq', [2, 128], F32R, kind='ExternalInput')
    wok = nc.dram_tensor('wok', [2, 128], F32R, kind='ExternalInput')
    wo = nc.dram_tensor('wo', [2, 128, D], F32R, kind='ExternalInput')
    # score-bias expansion tables (host-gathered from variable_bias / rtb)
    vones = nc.dram_tensor('vones', [1, 256], F32R, kind='ExternalInput')
    kstat = nc.dram_tensor('kstat', [32, N], F32R, kind='ExternalInput')
    qstat = nc.dram_tensor('qstat', [HPC, 48, N], F32R, kind='ExternalInput')
    apack = nc.dram_tensor('apack', [HPC, QC, 16, N], F32R,
                           kind='ExternalInput')
    if with_bias:
        bqr = nc.dram_tensor('bqr', [1, HPC * HD], F32R, kind='ExternalInput')
        bkr = nc.dram_tensor('bkr', [1, HPC * HD], F32R, kind='ExternalInput')
        bvr = nc.dram_tensor('bvr', [1, HPC * HD], F32R, kind='ExternalInput')
        boqr = nc.dram_tensor('boqr', [1, 128], F32R, kind='ExternalInput')
        bokr = nc.dram_tensor('bokr', [1, 128], F32R, kind='ExternalInput')
        onesd = nc.dram_tensor('onesd', [1, 512], F32R, kind='ExternalInput')
    out = nc.dram_tensor('out', [N, D], F32, kind='ExternalOutput')

    with tile.TileContext(nc) as tc:
        with tc.tile_pool(name='cst', bufs=1) as cst, \
             tc.tile_pool(name='big', bufs=1) as big, \
             tc.tile_pool(name='work', bufs=3) as work, \
             tc.tile_pool(name='et', bufs=3) as etp, \
             tc.tile_pool(name='ps2', bufs=2, space='PSUM') as ps2, \
             tc.tile_pool(name='po', bufs=2, space='PSUM') as po, \
             tc.tile_pool(name='pp', bufs=2, space='PSUM') as pp:

            # ---- SBUF tiles ----
            t_foT = cst.tile([2, N], F32R)
            t_woq = cst.tile([2, 128], F32R)
            t_wok = cst.tile([2, 128], F32R)
            t_wk = cst.tile([128, 4, HPC * HD], PDT)
            t_wq = cst.tile([128, 4, HPC * HD], PDT)
            t_wv = cst.tile([128, 4, HPC * HD], PDT)
            t_wo = cst.tile([128, 2, D], F32R)
            t_fhT = big.tile([128, 4, N], PDT)
            t_qp = [big.tile([128, N], F32R, tag=f'qp{hh}', name=f'qp{hh}')
                    for hh in range(HPC)]
            t_kp = [big.tile([128, N], F32R, tag=f'kp{hh}', name=f'kp{hh}')
                    for hh in range(HPC)]
            # v interleaved with denominator-ones per head
            t_vx = big.tile([128, KC, HPC, 2, 64], F32R, tag='vx', name='vx')
            t_ot = [big.tile([128, N], F32R, tag=f'ot{p}', name=f'ot{p}')
                    for p in range(2)]

            # ---- DMA order: tracks consumption order ----
            t_v1 = cst.tile([1, 256], F32R)
            nc.sync.dma_start(t_v1[:], vones[:])
            nc.sync.dma_start(t_foT[:], foT[:])
            nc.sync.dma_start(t_woq[:], woq[:])
            nc.sync.dma_start(t_wok[:], wok[:])
            fhT_r = fhT[:].rearrange('(o p) n -> p o n', p=128)
            wk_r = wk[:].rearrange('(o p) n -> p o n', p=128)
            wq_r = wq[:].rearrange('(o p) n -> p o n', p=128)
            nc.sync.dma_start(t_fhT[:], fhT_r[:])
            nc.sync.dma_start(t_wk[:], wk_r[:])
            nc.sync.dma_start(t_wq[:], wq_r[:])
            nc.sync.dma_start(t_wv[:], wv[:].rearrange('(o p) n -> p o n', p=128))
            nc.sync.dma_start(t_kp[0][80:112, :], kstat[:])
            nc.sync.dma_start(t_qp[0][80:128, 0:512], qstat[0][:, 0:512])
            nc.sync.dma_start(t_kp[0][112:128, :], apack[0, 0])
            nc.sync.dma_start(t_kp[1][80:112, :], kstat[:])
            nc.sync.dma_start(t_qp[1][80:128, 0:512], qstat[1][:, 0:512])
            nc.sync.dma_start(t_kp[1][112:128, :], apack[1, 0])
            for hh in range(2, HPC):
                nc.sync.dma_start(t_kp[hh][80:112, :], kstat[:])
                nc.sync.dma_start(t_qp[hh][80:128, 0:512],
                                  qstat[hh][:, 0:512])
                nc.sync.dma_start(t_kp[hh][112:128, :], apack[hh, 0])
            for hh in range(HPC):
                nc.sync.dma_start(t_qp[hh][80:128, 512:N],
                                  qstat[hh][:, 512:N])
            nc.sync.dma_start(t_wo[:], wo[:].rearrange('o p n -> p o n'))
            if with_bias:
                t_bq = cst.tile([1, HPC * HD], F32R)
                nc.sync.dma_start(t_bq[:], bqr[:])
                t_bk = cst.tile([1, HPC * HD], F32R)
                nc.sync.dma_start(t_bk[:], bkr[:])
                t_bv = cst.tile([1, HPC * HD], F32R)
                nc.sync.dma_start(t_bv[:], bvr[:])
                t_boq = cst.tile([1, 128], F32R)
                nc.sync.dma_start(t_boq[:], boqr[:])
                t_bok = cst.tile([1, 128], F32R)
                nc.sync.dma_start(t_bok[:], bokr[:])
                t_ones = cst.tile([1, 512], F32R)
                nc.sync.dma_start(t_ones[:], onesd[:])

            # ---- projection emitters (used pre-loop and as fillers) ----
            # gpsimd cannot touch PSUM on real HW (BIR verifier), so every
            # PSUM->SBUF copy is DVE, except the j0 obs copies which ride
            # the Activation engine's idle window before its first exp.
            # NOTE the asymmetry: oq->qp columns are QUERY-chunked (needed
            # only by slots of that j), but ok->kp columns are KEY-chunked
            # (every slot reads all of them from group 2 on), so the ok
            # sides must all land before the first slot's later groups.
            def emit_obs_side(j, which, act_eng=False):
                w_t = t_woq if which == 'oq' else t_wok
                pack = t_qp if which == 'oq' else t_kp
                p_o = pp.tile([128, 512], F32, tag='pp', name='p_o')
                nc.tensor.matmul(p_o[:], w_t[:],
                                 t_foT[:, j * 512:(j + 1) * 512],
                                 start=True, stop=(not with_bias))
                if with_bias:
                    bt = t_boq if which == 'oq' else t_bok
                    nc.tensor.matmul(p_o[:], bt[:], t_ones[:],
                                     start=False, stop=True)
                for hh in range(HPC):
                    dst = pack[hh][64:80, j * 512:(j + 1) * 512]
                    srcp = p_o[hh * 32:hh * 32 + OD, :]
                    on_act = act_eng is True or (act_eng == 'split' and hh < 2)
                    if on_act:
                        nc.scalar.copy(dst, srcp)
                    else:
                        nc.vector.tensor_copy(dst, srcp)

            def emit_qk(m, j, which):
                # q or k projection for head pair m, query chunk j
                w_t = t_wq if which == 'q' else t_wk
                pack = t_qp if which == 'q' else t_kp
                p_qt = pp.tile([128, 512], F32, tag='pp', name='p_qt')
                for kk in range(4):
                    nc.tensor.matmul(
                        p_qt[:], w_t[:, kk, m * 128:(m + 1) * 128],
                        t_fhT[:, kk, j * 512:(j + 1) * 512],
                        start=(kk == 0), stop=(not with_bias and kk == 3))
                if with_bias:
                    bt = t_bq if which == 'q' else t_bk
                    nc.tensor.matmul(p_qt[:], bt[:, m * 128:(m + 1) * 128],
                                     t_ones[:], start=False, stop=True)
                for s in range(2):
                    hh = 2 * m + s
                    nc.vector.tensor_copy(
                        pack[hh][0:64, j * 512:(j + 1) * 512],
                        p_qt[s * 64:(s + 1) * 64, :])

            def emit_v(kc, act_eng=False):
                # v projection for key chunk kc, plus the denominator-ones
                # block (written by a K=1 ones matmul so every byte of the
                # copy source is dependency-tracked); one strided copy
                # interleaves [v_h | ones] per head into t_vx
                p_v = pp.tile([128, 2, HPC * 64], F32, tag='pp', name='p_v')
                for kk in range(4):
                    nc.tensor.matmul(p_v[:, 0, :],
                                     t_fhT[:, kk, kc * 128:(kc + 1) * 128],
                                     t_wv[:, kk, :], start=(kk == 0),
                                     stop=(not with_bias and kk == 3))
                if with_bias:
                    nc.tensor.matmul(p_v[:, 0, :], t_ones[:, 0:128],
                                     t_bv[:], start=False, stop=True)
                nc.tensor.matmul(p_v[:, 1, :], t_v1[:, 0:128], t_v1[:],
                                 start=True, stop=True)
                src_ap = p_v[:].rearrange('p s (h c) -> p h s c', h=HPC)
                if act_eng:
                    nc.scalar.copy(t_vx[:, kc, :, :, :], src_ap)
                else:
                    nc.vector.tensor_copy(t_vx[:, kc, :, :, :], src_ap)

            def emit_y_qq(j, qq, t_y):
                # one 128-row block of the partial out-projection for query
                # chunk j; the last chunk's copies ride the Activation
                # engine (idle after its final exp) to shorten the tail
                qc = 4 * j + qq
                p_y = pp.tile([128, D], F32, tag='pp', name='p_y')
                for p in range(2):
                    nc.tensor.matmul(p_y[:],
                                     t_ot[p][:, qc * 128:(qc + 1) * 128],
                                     t_wo[:, p, :], start=(p == 0),
                                     stop=(p == 1))
                if j == QC - 1:
                    nc.scalar.copy(t_y[:, qq, :], p_y[:])
                else:
                    nc.vector.tensor_copy(t_y[:, qq, :], p_y[:])
                nc.sync.dma_start(out[qc * 128:(qc + 1) * 128, :],
                                  t_y[:, qq, :])

            # ---- pre-loop: only what slot 0 group 0 needs early ----
            emit_obs_side(0, 'oq', act_eng=True)
            emit_obs_side(0, 'ok', act_eng=True)
            emit_obs_side(1, 'ok')
            emit_qk(0, 0, 'k')
            emit_qk(0, 0, 'q')

            # fillers: (deadline_group_idx, closure), deadline-sorted, in
            # true readiness order. ~3 groups of slack for the psum->copy
            # chain latency ahead of each consumer.
            fillers = [
                (1, lambda: emit_v(0, act_eng=True)),
                (1, lambda: emit_v(1, act_eng=True)),
                (2, lambda: emit_qk(0, 1, 'k')),       # keys 512:1024 at g2
                (2, lambda: emit_v(2, act_eng=True)),
                (2, lambda: emit_v(3, act_eng=True)),
                (3, lambda: emit_qk(0, 2, 'k')),       # keys 1024:1536 at g4
                (3, lambda: emit_v(4, act_eng=True)),
                (3, lambda: emit_v(5, act_eng=True)),
                (3, lambda: emit_obs_side(2, 'ok', act_eng='split')),
                (4, lambda: emit_v(6)), (4, lambda: emit_v(7)),
                (5, lambda: emit_v(8)), (5, lambda: emit_v(9)),
                (6, lambda: emit_v(10)), (6, lambda: emit_v(11)),
                (6, lambda: emit_qk(1, 0, 'k')),       # slot 2 = (j0, h2)
                (7, lambda: emit_qk(1, 0, 'q')),
                (8, lambda: emit_qk(1, 1, 'k')),
                (10, lambda: emit_qk(1, 2, 'k')),
                (14, lambda: emit_obs_side(1, 'oq')),  # slot 4 = (j1, h0)
                (15, lambda: emit_qk(0, 1, 'q')),
                (28, lambda: emit_qk(1, 1, 'q')),      # slot 6 = (j1, h2)
                (38, lambda: emit_obs_side(2, 'oq')),  # slot 8 = (j2, h0)
                (39, lambda: emit_qk(0, 2, 'q')),
                (52, lambda: emit_qk(1, 2, 'q')),      # slot 10 = (j2, h2)
            ]
            fi = 0

            # ---- main attention loop: score/exp/AV pipelined one group ----
            seq = [(si, g) for si in range(len(SLOTS)) for g in range(GPS)]
            n_groups = len(seq)
            pend = None          # (slot_idx, g, p_ot, t_et) awaiting AV

            def emit_scores(si, g):
                j, hh = SLOTS[si]
                p_s = ps2.tile([128, 2, 512], F32, tag='sc', name='p_s')
                mm = None
                for i2 in range(2):
                    kc = 2 * g + i2
                    mm = nc.tensor.matmul(
                        p_s[:, i2, :],
                        t_kp[hh][:, kc * 128:(kc + 1) * 128],
                        t_qp[hh][:, j * 512:(j + 1) * 512],
                        start=True, stop=True)
                t_et = etp.tile([128, 2, 512], F32R, tag='et', name='t_et')
                nc.scalar.activation(t_et[:], p_s[:], EXP)
                return t_et, mm

            def emit_av(si, g, p_ot, t_et):
                j, hh = SLOTS[si]
                for i2 in range(2):
                    kc = 2 * g + i2
                    nc.tensor.matmul(
                        p_ot[:],
                        t_vx[:, kc, hh].rearrange('p a c -> p (a c)'),
                        t_et[:, i2, :],
                        start=(kc == 0), stop=(kc == KC - 1))

            pend_y = []

            def finish_slot(si, p_ot):
                j, hh = SLOTS[si]
                t_rec = work.tile([64, 512], F32, tag='rec', name='t_rec')
                nc.vector.reciprocal(t_rec[:], p_ot[64:128, :])
                nc.vector.tensor_mul(
                    t_ot[hh // 2][(hh % 2) * 64:(hh % 2) * 64 + 64,
                                  j * 512:(j + 1) * 512],
                    p_ot[0:64, :], t_rec[:])
                if hh == HPC - 1:
                    # spread the 4 out-projection blocks across the next
                    # groups so they don't wall off the next slot's scores
                    t_y = work.tile([128, 4, D], F32, tag='y', name='t_y')
                    for qq in range(4):
                        pend_y.append((j, qq, t_y))

            p_ot_by_slot = {}
            for idx in range(n_groups):
                si, g = seq[idx]
                while fi < len(fillers) and fillers[fi][0] <= idx:
                    fillers[fi][1]()
                    fi += 1
                if g == 0:
                    p_ot_by_slot[si] = po.tile([128, 512], F32, tag='po',
                                               name='p_ot')
                p_ot_cur = p_ot_by_slot[si]
                t_et, last_mm = emit_scores(si, g)
                if g == GPS - 1:
                    # next j for this head reuses kp rows 112:128. Tile does
                    # NOT track the DMA-write-after-PE-read hazard here (the
                    # reload only gets DMA-queue-ring waits), so add the WAR
                    # edge on this slot's last score matmul explicitly: the
                    # PE stream is in-order, so that one edge covers all 12
                    # score reads of the old rows.
                    j, hh = SLOTS[si]
                    if j + 1 < QC:
                        # issue from the otherwise-idle gpsimd engine: the
                        # WAR wait must not block SP's DMA stream (it would
                        # hold up y-output DMAs and cross-couple queues)
                        dma = nc.gpsimd.dma_start(t_kp[hh][112:128, :],
                                                  apack[hh, j + 1])
                        deps = InstructionNameOrderedSet()
                        deps.add(last_mm.ins.name)
                        dma.ins.add_sync_dependencies_from(deps)
                if pend is not None:
                    emit_av(*pend)
                    if pend[1] == GPS - 1:
                        finish_slot(pend[0], pend[2])
                if pend_y:
                    emit_y_qq(*pend_y.pop(0))
                pend = (si, g, p_ot_cur, t_et)
            emit_av(*pend)
            finish_slot(pend[0], pend[2])
            while pend_y:
                emit_y_qq(*pend_y.pop(0))

    nc._state.remove_inst_callback(_chain_cb)
    _split_waits(nc)
    return nc


_NC_CACHE = {}


def _get_nc(with_bias=False):
    if with_bias not in _NC_CACHE:
        _NC_CACHE[with_bias] = _build(with_bias)
    return _NC_CACHE[with_bias]


def _pad_obs(a):
    # lay each head's 16 obs channels at a 32-column boundary (PSUM reads
    # must start at 32-partition-aligned offsets)
    out = np.zeros((a.shape[0], 128), np.float32)
    for hh in range(HPC):
        out[:, hh * 32:hh * 32 + OD] = a[:, hh * OD:(hh + 1) * OD]
    return out


def _host_prep(h, observation_state, Wq, bq, Wk, bk, Wv, bv, Wo, bo,
               Woq, boq, Wok, bok, variable_bias, relative_time_bias,
               with_bias=False):
    f32 = np.float32
    h = np.asarray(h, f32)
    obs = np.asarray(observation_state, f32)
    Kidx = np.arange(N)
    tK = Kidx // V                                     # time bin of each token
    kstat = (Kidx[None, :] % V == np.arange(V)[:, None]).astype(f32)
    bq16 = ((Kidx[None, :] // V) % 16 == np.arange(16)[:, None]).astype(f32)

    in_maps = []
    for c in range(NCORES):
        b, hg = divmod(c, 2)
        h0 = hg * HPC
        cs, ce = h0 * HD, (h0 + HPC) * HD
        os_, oe = h0 * OD, (h0 + HPC) * OD
        qstat = np.empty((HPC, 48, N), f32)
        ap = np.empty((HPC, QC, 16, N), f32)
        for hh in range(HPC):
            head = h0 + hh
            vb = np.asarray(variable_bias[head], f32)
            rtb = np.asarray(relative_time_bias[head], f32)
            qstat[hh, :V] = vb[Kidx % V, :].T          # VB_h[Q%32, j]
            qstat[hh, V:] = bq16
            for j in range(QC):
                # A_hj[s, K] = rtb[16j + s - K//32 + 47]
                idx = 16 * j + np.arange(16)[:, None] - tK[None, :] + (T - 1)
                ap[hh, j] = rtb[idx]
        pdt = ml_dtypes.bfloat16 if PROJ_BF16 else f32
        m = {
            'fhT': np.ascontiguousarray(h[b].reshape(N, D).T).astype(pdt),
            'foT': np.ascontiguousarray(obs[b].reshape(N, 2).T),
            # content / obs scales folded into the q-side weights
            'wq': np.ascontiguousarray(
                np.asarray(Wq, f32)[:, cs:ce] * SCALE).astype(pdt),
            'wk': np.ascontiguousarray(
                np.asarray(Wk, f32)[:, cs:ce]).astype(pdt),
            'wv': np.ascontiguousarray(
                np.asarray(Wv, f32)[:, cs:ce]).astype(pdt),
            'woq': _pad_obs(np.asarray(Woq, f32)[:, os_:oe] * OBS_SCALE),
            'wok': _pad_obs(np.asarray(Wok, f32)[:, os_:oe]),
            'wo': np.ascontiguousarray(
                np.asarray(Wo, f32)[cs:ce, :].reshape(2, 128, D)),
            'vones': np.ones((1, 256), f32),
            'kstat': kstat,
            'qstat': qstat,
            'apack': ap,
        }
        if with_bias:
            m.update({
                'bqr': np.ascontiguousarray(
                    np.asarray(bq, f32)[None, cs:ce] * SCALE),
                'bkr': np.ascontiguousarray(np.asarray(bk, f32)[None, cs:ce]),
                'bvr': np.ascontiguousarray(np.asarray(bv, f32)[None, cs:ce]),
                'boqr': _pad_obs(np.asarray(boq, f32)[None, os_:oe] * OBS_SCALE),
                'bokr': _pad_obs(np.asarray(bok, f32)[None, os_:oe]),
                'onesd': np.ones((1, 512), f32),
            })
        in_maps.append(m)
    return in_maps


def kernel(**inputs):
    with_bias = any(
        np.any(np.asarray(inputs[k])) for k in ('bq', 'bk', 'bv', 'boq', 'bok'))
    nc = _get_nc(with_bias)
    in_maps = _host_prep(**inputs, with_bias=with_bias)
    res = run_bass_kernel_spmd(nc, in_maps, core_ids=list(range(NCORES)))
    bo = np.asarray(inputs['bo'], np.float32)
    outf = np.zeros((B, N, D), np.float32)
    for c in range(NCORES):
        outf[c // 2] += res.results[c]['out']
    outf += bo[None, None, :]
    return outf.reshape(B, T, V, D)


# revision 11
# speedup vs baseline: 1.0203x; 1.0203x over previous
"""Trainium2 Bass kernel for ClinicalStateFormationOperator (v2, pipelined).

Full-input contract: kernel(**inputs) takes the complete (unsharded) numpy
inputs and returns the full [B, T, V, D] output. Work is sharded across 8
NeuronCores as (batch, head-group): core c handles batch c//2 and heads
(c%2)*4 .. (c%2)*4+3. Each core computes its 4 heads' attention and the
partial output projection; the host sums the two partial projections per
batch and adds the output bias.

Math identical to v1 (scores computed transposed with a packed K=128
contraction: content 64 | obs 16 | variable-bias one-hots 32 | time-bias
basis 16; E^T = exp(scores^T); [out^T; denom] = [v_h | ones]^T @ E^T;
divide; partial out-projection). 1/sqrt(hd) and 1/sqrt(od) are folded into
Wq/Woq host-side so all PSUM->SBUF moves are plain copies.

Scheduling (the point of v2): the score->exp->AV chain is software-
pipelined one group ahead (PE emits scores for group i+1 before the AV of
group i), the Activation engine runs the exps plus the early-phase copies
that fit its pre-steady-state idle windows, all other PSUM->SBUF copies
run on DVE (gpsimd cannot access PSUM), and the projection matmuls are
emitted as deadline-tagged fillers inside the attention loop. Per-engine
execution order is pinned to emission order with nosync dependency chains
(see _chain_cb) because the Tile scheduler's region tracker only gives the
first-emitted reader of a region an edge to its writer.

v is stored interleaved [128, KC, head, {v|ones}, 64] so each head's AV
stationary operand [v_h | ones] is one contiguous 128-wide block and each
key chunk needs only a single strided copy from the projection PSUM.

PSUM: 4 banks score double-buffer ([128,2,512] x2) + 2 banks attention-out
([128,512] x2) + 2 banks projections/y ([128,512] x2).
"""

import ml_dtypes
import numpy as np

import concourse.bass as bass
import concourse.mybir as mybir
import concourse.tile as tile
from concourse.bass_utils import run_bass_kernel_spmd
from concourse.instruction_name_ordered_set import InstructionNameOrderedSet

V = 32
T = 48
D = 512
H = 8
HD = D // H          # 64
OD = 16
B = 4
N = T * V            # 1536
HPC = 4              # heads per core
NCORES = 8
SCALE = 1.0 / np.sqrt(HD)
OBS_SCALE = 1.0 / np.sqrt(OD)

F32 = mybir.dt.float32
F32R = mybir.dt.float32r
BF16 = mybir.dt.bfloat16
EXP = mybir.ActivationFunctionType.Exp

# bf16 q/k/v projection inputs: halves the startup DMA (which gates the
# first ~25 groups) for ~1e-2 worst-case relative error, well under the
# 2e-2 gate. Score packs, biases and the out-projection stay float32r.
PROJ_BF16 = True
PDT = BF16 if PROJ_BF16 else F32R

KC = N // 128        # 12 key chunks of 128
QC = N // 512        # 3 query chunks of 512
GPS = 6              # score/exp/AV groups per (head, qchunk): 2 key chunks

# slot order: j outer, h inner (v1 order). All slots need all keys, so the
# hard-early deliveries are fhT + k-proj(m0) only; q/obs/apack columns for
# j1/j2 and the m=1 packs have 12+ groups of slack.
SLOTS = [(j, h) for j in range(QC) for h in range(HPC)]


def _split_waits(nc, max_waits=1):
    """Walrus in this container allows only one sync-wait slot per
    instruction; spill extra waits onto preceding same-engine NoOps."""
    def fix_bb(bb):
        changed = False
        new = []
        for inst in bb.instructions:
            si = inst.sync_info
            if si is not None and len(si.on_wait) > max_waits:
                waits = list(si.on_wait)
                for w in waits[:-max_waits]:
                    new.append(mybir.InstNoOp(
                        name=nc.get_next_instruction_name(),
                        engine=inst.engine, ins=[], outs=[],
                        sync_info=mybir.SyncInfo(on_wait=[w], on_update=[])))
                    changed = True
                si.on_wait = waits[-max_waits:]
            new.append(inst)
        if changed:
            bb.instructions = new
        for sub in getattr(bb, 'blocks', []) or []:
            fix_bb(sub)
    for f in nc.m.functions:
        for bb in f.blocks:
            fix_bb(bb)


def _build(with_bias=False):
    nc = bass.Bass()

    # The Tile scheduler reorders same-engine instructions while its region
    # tracker gives only the FIRST-emitted reader of a region an edge to the
    # writer; a later reader hoisted above that first reader can then run
    # before the data lands (observed: score matmuls reading uninitialized
    # pack rows). Chain every compute instruction to its same-engine
    # predecessor so the per-engine order is exactly emission order, which
    # restores the transitivity the tracker's dedup assumes.
    _chain_last = {}
    _chain_types = (mybir.InstMatmult, mybir.InstTensorCopy,
                    mybir.InstActivation, mybir.InstReciprocal,
                    mybir.InstTensorTensor)
    _chain_engines = (mybir.EngineType.PE, mybir.EngineType.DVE,
                      mybir.EngineType.Activation)
    def _chain_cb(inst):
        if inst.engine in _chain_engines and isinstance(inst, _chain_types):
            prev = _chain_last.get(inst.engine)
            if prev is not None:
                deps = InstructionNameOrderedSet()
                deps.add(prev.name)
                # nosync: same-engine ordering is free (in-order engines);
                # a sync dep here would cost a redundant semaphore wait
                inst.add_nosync_dependencies_from(deps)
            _chain_last[inst.engine] = inst
    nc._state.push_inst_callback(_chain_cb)

    # ---- per-core DRAM I/O (data differs per core, program is SPMD) ----
    fhT = nc.dram_tensor('fhT', [D, N], PDT, kind='ExternalInput')
    foT = nc.dram_tensor('foT', [2, N], F32R, kind='ExternalInput')
    wq = nc.dram_tensor('wq', [D, HPC * HD], PDT, kind='ExternalInput')
    wk = nc.dram_tensor('wk', [D, HPC * HD], PDT, kind='ExternalInput')
    wv = nc.dram_tensor('wv', [D, HPC * HD], PDT, kind='ExternalInput')
    woq = nc.dram_tensor('woq', [2, 128], F32R, kind='ExternalInput')
    wok = nc.dram_tensor('wok', [2, 128], F32R, kind='ExternalInput')
    wo = nc.dram_tensor('wo', [2, 128, D], F32R, kind='ExternalInput')
    # score-bias expansion tables (host-gathered from variable_bias / rtb)
    vones = nc.dram_tensor('vones', [1, 256], F32R, kind='ExternalInput')
    kstat = nc.dram_tensor('kstat', [32, N], F32R, kind='ExternalInput')
    qstat = nc.dram_tensor('qstat', [HPC, 48, N], F32R, kind='ExternalInput')
    apack = nc.dram_tensor('apack', [HPC, QC, 16, N], F32R,
                           kind='ExternalInput')
    if with_bias:
        bqr = nc.dram_tensor('bqr', [1, HPC * HD], F32R, kind='ExternalInput')
        bkr = nc.dram_tensor('bkr', [1, HPC * HD], F32R, kind='ExternalInput')
        bvr = nc.dram_tensor('bvr', [1, HPC * HD], F32R, kind='ExternalInput')
        boqr = nc.dram_tensor('boqr', [1, 128], F32R, kind='ExternalInput')
        bokr = nc.dram_tensor('bokr', [1, 128], F32R, kind='ExternalInput')
        onesd = nc.dram_tensor('onesd', [1, 512], F32R, kind='ExternalInput')
    out = nc.dram_tensor('out', [N, D], F32, kind='ExternalOutput')

    with tile.TileContext(nc) as tc:
        with tc.tile_pool(name='cst', bufs=1) as cst, \
             tc.tile_pool(name='big', bufs=1) as big, \
             tc.tile_pool(name='work', bufs=3) as work, \
             tc.tile_pool(name='et', bufs=3) as etp, \
             tc.tile_pool(name='ps2', bufs=2, space='PSUM') as ps2, \
             tc.tile_pool(name='po', bufs=2, space='PSUM') as po, \
             tc.tile_pool(name='pp', bufs=2, space='PSUM') as pp:

            # ---- SBUF tiles ----
            t_foT = cst.tile([2, N], F32R)
            t_woq = cst.tile([2, 128], F32R)
            t_wok = cst.tile([2, 128], F32R)
            t_wk = cst.tile([128, 4, HPC * HD], PDT)
            t_wq = cst.tile([128, 4, HPC * HD], PDT)
            t_wv = cst.tile([128, 4, HPC * HD], PDT)
            t_wo = cst.tile([128, 2, D], F32R)
            t_fhT = big.tile([128, 4, N], PDT)
            t_qp = [big.tile([128, N], F32R, tag=f'qp{hh}', name=f'qp{hh}')
                    for hh in range(HPC)]
            t_kp = [big.tile([128, N], F32R, tag=f'kp{hh}', name=f'kp{hh}')
                    for hh in range(HPC)]
            # v interleaved with denominator-ones per head
            t_vx = big.tile([128, KC, HPC, 2, 64], F32R, tag='vx', name='vx')
            t_ot = [big.tile([128, N], F32R, tag=f'ot{p}', name=f'ot{p}')
                    for p in range(2)]

            # ---- DMA order: tracks consumption order ----
            t_v1 = cst.tile([1, 256], F32R)
            nc.sync.dma_start(t_v1[:], vones[:])
            nc.sync.dma_start(t_foT[:], foT[:])
            nc.sync.dma_start(t_woq[:], woq[:])
            nc.sync.dma_start(t_wok[:], wok[:])
            fhT_r = fhT[:].rearrange('(o p) n -> p o n', p=128)
            wk_r = wk[:].rearrange('(o p) n -> p o n', p=128)
            wq_r = wq[:].rearrange('(o p) n -> p o n', p=128)
            nc.sync.dma_start(t_fhT[:], fhT_r[:])
            nc.sync.dma_start(t_wk[:], wk_r[:])
            nc.sync.dma_start(t_wq[:], wq_r[:])
            nc.sync.dma_start(t_wv[:], wv[:].rearrange('(o p) n -> p o n', p=128))
            nc.sync.dma_start(t_kp[0][80:112, :], kstat[:])
            nc.sync.dma_start(t_qp[0][80:128, 0:512], qstat[0][:, 0:512])
            nc.sync.dma_start(t_kp[0][112:128, :], apack[0, 0])
            nc.sync.dma_start(t_kp[1][80:112, :], kstat[:])
            nc.sync.dma_start(t_qp[1][80:128, 0:512], qstat[1][:, 0:512])
            nc.sync.dma_start(t_kp[1][112:128, :], apack[1, 0])
            for hh in range(2, HPC):
                nc.sync.dma_start(t_kp[hh][80:112, :], kstat[:])
                nc.sync.dma_start(t_qp[hh][80:128, 0:512],
                                  qstat[hh][:, 0:512])
                nc.sync.dma_start(t_kp[hh][112:128, :], apack[hh, 0])
            for hh in range(HPC):
                nc.sync.dma_start(t_qp[hh][80:128, 512:N],
                                  qstat[hh][:, 512:N])
            nc.sync.dma_start(t_wo[:], wo[:].rearrange('o p n -> p o n'))
            if with_bias:
                t_bq = cst.tile([1, HPC * HD], F32R)
                nc.sync.dma_start(t_bq[:], bqr[:])
                t_bk = cst.tile([1, HPC * HD], F32R)
                nc.sync.dma_start(t_bk[:], bkr[:])
                t_bv = cst.tile([1, HPC * HD], F32R)
                nc.sync.dma_start(t_bv[:], bvr[:])
                t_boq = cst.tile([1, 128], F32R)
                nc.sync.dma_start(t_boq[:], boqr[:])
                t_bok = cst.tile([1, 128], F32R)
                nc.sync.dma_start(t_bok[:], bokr[:])
                t_ones = cst.tile([1, 512], F32R)
                nc.sync.dma_start(t_ones[:], onesd[:])

            # ---- projection emitters (used pre-loop and as fillers) ----
            # gpsimd cannot touch PSUM on real HW (BIR verifier), so every
            # PSUM->SBUF copy is DVE, except the j0 obs copies which ride
            # the Activation engine's idle window before its first exp.
            # NOTE the asymmetry: oq->qp columns are QUERY-chunked (needed
            # only by slots of that j), but ok->kp columns are KEY-chunked
            # (every slot reads all of them from group 2 on), so the ok
            # sides must all land before the first slot's later groups.
            def emit_obs_side(j, which, act_eng=False):
                w_t = t_woq if which == 'oq' else t_wok
                pack = t_qp if which == 'oq' else t_kp
                p_o = pp.tile([128, 512], F32, tag='pp', name='p_o')
                nc.tensor.matmul(p_o[:], w_t[:],
                                 t_foT[:, j * 512:(j + 1) * 512],
                                 start=True, stop=(not with_bias))
                if with_bias:
                    bt = t_boq if which == 'oq' else t_bok
                    nc.tensor.matmul(p_o[:], bt[:], t_ones[:],
                                     start=False, stop=True)
                for hh in range(HPC):
                    dst = pack[hh][64:80, j * 512:(j + 1) * 512]
                    srcp = p_o[hh * 32:hh * 32 + OD, :]
                    on_act = act_eng is True or (act_eng == 'split' and hh < 2)
                    if on_act:
                        nc.scalar.copy(dst, srcp)
                    else:
                        nc.vector.tensor_copy(dst, srcp)

            def emit_qk(m, j, which):
                # q or k projection for head pair m, query chunk j
                w_t = t_wq if which == 'q' else t_wk
                pack = t_qp if which == 'q' else t_kp
                p_qt = pp.tile([128, 512], F32, tag='pp', name='p_qt')
                for kk in range(4):
                    nc.tensor.matmul(
                        p_qt[:], w_t[:, kk, m * 128:(m + 1) * 128],
                        t_fhT[:, kk, j * 512:(j + 1) * 512],
                        start=(kk == 0), stop=(not with_bias and kk == 3))
                if with_bias:
                    bt = t_bq if which == 'q' else t_bk
                    nc.tensor.matmul(p_qt[:], bt[:, m * 128:(m + 1) * 128],
                                     t_ones[:], start=False, stop=True)
                for s in range(2):
                    hh = 2 * m + s
                    nc.vector.tensor_copy(
                        pack[hh][0:64, j * 512:(j + 1) * 512],
                        p_qt[s * 64:(s + 1) * 64, :])

            def emit_v(kc, act_eng=False):
                # v projection for key chunk kc, plus the denominator-ones
                # block (written by a K=1 ones matmul so every byte of the
                # copy source is dependency-tracked); one strided copy
                # interleaves [v_h | ones] per head into t_vx
                p_v = pp.tile([128, 2, HPC * 64], F32, tag='pp', name='p_v')
                for kk in range(4):
                    nc.tensor.matmul(p_v[:, 0, :],
                                     t_fhT[:, kk, kc * 128:(kc + 1) * 128],
                                     t_wv[:, kk, :], start=(kk == 0),
                                     stop=(not with_bias and kk == 3))
                if with_bias:
                    nc.tensor.matmul(p_v[:, 0, :], t_ones[:, 0:128],
                                     t_bv[:], start=False, stop=True)
                nc.tensor.matmul(p_v[:, 1, :], t_v1[:, 0:128], t_v1[:],
                                 start=True, stop=True)
                src_ap = p_v[:].rearrange('p s (h c) -> p h s c', h=HPC)
                if act_eng:
                    nc.scalar.copy(t_vx[:, kc, :, :, :], src_ap)
                else:
                    nc.vector.tensor_copy(t_vx[:, kc, :, :, :], src_ap)

            def emit_y_qq(j, qq, t_y):
                # one 128-row block of the partial out-projection for query
                # chunk j; the last chunk's copies ride the Activation
                # engine (idle after its final exp) to shorten the tail
                qc = 4 * j + qq
                p_y = pp.tile([128, D], F32, tag='pp', name='p_y')
                for p in range(2):
                    nc.tensor.matmul(p_y[:],
                                     t_ot[p][:, qc * 128:(qc + 1) * 128],
                                     t_wo[:, p, :], start=(p == 0),
                                     stop=(p == 1))
                if j == QC - 1:
                    nc.scalar.copy(t_y[:, qq, :], p_y[:])
                else:
                    nc.vector.tensor_copy(t_y[:, qq, :], p_y[:])
                nc.sync.dma_start(out[qc * 128:(qc + 1) * 128, :],
                                  t_y[:, qq, :])

            # ---- pre-loop: only what slot 0 group 0 needs early ----
            emit_obs_side(0, 'oq', act_eng=True)
            emit_obs_side(0, 'ok', act_eng=True)
            emit_obs_side(1, 'ok')
            emit_qk(0, 0, 'k')
            emit_qk(0, 0, 'q')

            # fillers: (deadline_group_idx, closure), deadline-sorted, in
            # true readiness order. ~3 groups of slack for the psum->copy
            # chain latency ahead of each consumer.
            fillers = [
                (1, lambda: emit_v(0, act_eng=True)),
                (1, lambda: emit_v(1, act_eng=True)),
                (2, lambda: emit_qk(0, 1, 'k')),       # keys 512:1024 at g2
                (2, lambda: emit_v(2, act_eng=True)),
                (2, lambda: emit_v(3, act_eng=True)),
                (3, lambda: emit_qk(0, 2, 'k')),       # keys 1024:1536 at g4
                (3, lambda: emit_v(4, act_eng=True)),
                (3, lambda: emit_v(5, act_eng=True)),
                (3, lambda: emit_obs_side(2, 'ok', act_eng='split')),
                (4, lambda: emit_v(6)), (4, lambda: emit_v(7)),
                (5, lambda: emit_v(8)), (5, lambda: emit_v(9)),
                (6, lambda: emit_v(10)), (6, lambda: emit_v(11)),
                (6, lambda: emit_qk(1, 0, 'k')),       # slot 2 = (j0, h2)
                (7, lambda: emit_qk(1, 0, 'q')),
                (8, lambda: emit_qk(1, 1, 'k')),
                (10, lambda: emit_qk(1, 2, 'k')),
                (14, lambda: emit_obs_side(1, 'oq')),  # slot 4 = (j1, h0)
                (15, lambda: emit_qk(0, 1, 'q')),
                (28, lambda: emit_qk(1, 1, 'q')),      # slot 6 = (j1, h2)
                (38, lambda: emit_obs_side(2, 'oq')),  # slot 8 = (j2, h0)
                (39, lambda: emit_qk(0, 2, 'q')),
                (52, lambda: emit_qk(1, 2, 'q')),      # slot 10 = (j2, h2)
            ]
            fi = 0

            # ---- main attention loop: score/exp/AV pipelined one group ----
            seq = [(si, g) for si in range(len(SLOTS)) for g in range(GPS)]
            n_groups = len(seq)
            pend = None          # (slot_idx, g, p_ot, t_et) awaiting AV

            def emit_scores(si, g):
                j, hh = SLOTS[si]
                p_s = ps2.tile([128, 2, 512], F32, tag='sc', name='p_s')
                mm = None
                for i2 in range(2):
                    kc = 2 * g + i2
                    mm = nc.tensor.matmul(
                        p_s[:, i2, :],
                        t_kp[hh][:, kc * 128:(kc + 1) * 128],
                        t_qp[hh][:, j * 512:(j + 1) * 512],
                        start=True, stop=True)
                t_et = etp.tile([128, 2, 512], F32R, tag='et', name='t_et')
                nc.scalar.activation(t_et[:], p_s[:], EXP)
                return t_et, mm

            def emit_av(si, g, p_ot, t_et):
                j, hh = SLOTS[si]
                for i2 in range(2):
                    kc = 2 * g + i2
                    nc.tensor.matmul(
                        p_ot[:],
                        t_vx[:, kc, hh].rearrange('p a c -> p (a c)'),
                        t_et[:, i2, :],
                        start=(kc == 0), stop=(kc == KC - 1))

            pend_y = []

            def finish_slot(si, p_ot):
                j, hh = SLOTS[si]
                t_rec = work.tile([64, 512], F32, tag='rec', name='t_rec')
                nc.vector.reciprocal(t_rec[:], p_ot[64:128, :])
                nc.vector.tensor_mul(
                    t_ot[hh // 2][(hh % 2) * 64:(hh % 2) * 64 + 64,
                                  j * 512:(j + 1) * 512],
                    p_ot[0:64, :], t_rec[:])
                if hh == HPC - 1:
                    # spread the 4 out-projection blocks across the next
                    # groups so they don't wall off the next slot's scores
                    t_y = work.tile([128, 4, D], F32, tag='y', name='t_y')
                    for qq in range(4):
                        pend_y.append((j, qq, t_y))

            p_ot_by_slot = {}
            y_skip = 0
            for idx in range(n_groups):
                si, g = seq[idx]
                while fi < len(fillers) and fillers[fi][0] <= idx:
                    fillers[fi][1]()
                    fi += 1
                if g == 0:
                    p_ot_by_slot[si] = po.tile([128, 512], F32, tag='po',
                                               name='p_ot')
                p_ot_cur = p_ot_by_slot[si]
                t_et, last_mm = emit_scores(si, g)
                if g == GPS - 1:
                    # next j for this head reuses kp rows 112:128. Tile does
                    # NOT track the DMA-write-after-PE-read hazard here (the
                    # reload only gets DMA-queue-ring waits), so add the WAR
                    # edge on this slot's last score matmul explicitly: the
                    # PE stream is in-order, so that one edge covers all 12
                    # score reads of the old rows.
                    j, hh = SLOTS[si]
                    if j + 1 < QC:
                        # issue from the otherwise-idle gpsimd engine: the
                        # WAR wait must not block SP's DMA stream (it would
                        # hold up y-output DMAs and cross-couple queues)
                        dma = nc.gpsimd.dma_start(t_kp[hh][112:128, :],
                                                  apack[hh, j + 1])
                        deps = InstructionNameOrderedSet()
                        deps.add(last_mm.ins.name)
                        dma.ins.add_sync_dependencies_from(deps)
                if pend is not None:
                    emit_av(*pend)
                    if pend[1] == GPS - 1:
                        finish_slot(pend[0], pend[2])
                        if pend_y:
                            # let the divide (recip+mul) clear the DVE chain
                            # before the first y-projection reads t_ot, and
                            # spread the blocks so scores aren't walled off
                            y_skip = 2
                if pend_y:
                    if y_skip > 0:
                        y_skip -= 1
                    elif idx % 2 == 0:
                        emit_y_qq(*pend_y.pop(0))
                pend = (si, g, p_ot_cur, t_et)
            emit_av(*pend)
            finish_slot(pend[0], pend[2])
            while pend_y:
                emit_y_qq(*pend_y.pop(0))

    nc._state.remove_inst_callback(_chain_cb)
    _split_waits(nc)
    return nc


_NC_CACHE = {}


def _get_nc(with_bias=False):
    if with_bias not in _NC_CACHE:
        _NC_CACHE[with_bias] = _build(with_bias)
    return _NC_CACHE[with_bias]


def _pad_obs(a):
    # lay each head's 16 obs channels at a 32-column boundary (PSUM reads
    # must start at 32-partition-aligned offsets)
    out = np.zeros((a.shape[0], 128), np.float32)
    for hh in range(HPC):
        out[:, hh * 32:hh * 32 + OD] = a[:, hh * OD:(hh + 1) * OD]
    return out


def _host_prep(h, observation_state, Wq, bq, Wk, bk, Wv, bv, Wo, bo,
               Woq, boq, Wok, bok, variable_bias, relative_time_bias,
               with_bias=False):
    f32 = np.float32
    h = np.asarray(h, f32)
    obs = np.asarray(observation_state, f32)
    Kidx = np.arange(N)
    tK = Kidx // V                                     # time bin of each token
    kstat = (Kidx[None, :] % V == np.arange(V)[:, None]).astype(f32)
    bq16 = ((Kidx[None, :] // V) % 16 == np.arange(16)[:, None]).astype(f32)

    in_maps = []
    for c in range(NCORES):
        b, hg = divmod(c, 2)
        h0 = hg * HPC
        cs, ce = h0 * HD, (h0 + HPC) * HD
        os_, oe = h0 * OD, (h0 + HPC) * OD
        qstat = np.empty((HPC, 48, N), f32)
        ap = np.empty((HPC, QC, 16, N), f32)
        for hh in range(HPC):
            head = h0 + hh
            vb = np.asarray(variable_bias[head], f32)
            rtb = np.asarray(relative_time_bias[head], f32)
            qstat[hh, :V] = vb[Kidx % V, :].T          # VB_h[Q%32, j]
            qstat[hh, V:] = bq16
            for j in range(QC):
                # A_hj[s, K] = rtb[16j + s - K//32 + 47]
                idx = 16 * j + np.arange(16)[:, None] - tK[None, :] + (T - 1)
                ap[hh, j] = rtb[idx]
        pdt = ml_dtypes.bfloat16 if PROJ_BF16 else f32
        m = {
            'fhT': np.ascontiguousarray(h[b].reshape(N, D).T).astype(pdt),
            'foT': np.ascontiguousarray(obs[b].reshape(N, 2).T),
            # content / obs scales folded into the q-side weights
            'wq': np.ascontiguousarray(
                np.asarray(Wq, f32)[:, cs:ce] * SCALE).astype(pdt),
            'wk': np.ascontiguousarray(
                np.asarray(Wk, f32)[:, cs:ce]).astype(pdt),
            'wv': np.ascontiguousarray(
                np.asarray(Wv, f32)[:, cs:ce]).astype(pdt),
            'woq': _pad_obs(np.asarray(Woq, f32)[:, os_:oe] * OBS_SCALE),
            'wok': _pad_obs(np.asarray(Wok, f32)[:, os_:oe]),
            'wo': np.ascontiguousarray(
                np.asarray(Wo, f32)[cs:ce, :].reshape(2, 128, D)),
            'vones': np.ones((1, 256), f32),
            'kstat': kstat,
            'qstat': qstat,
            'apack': ap,
        }
        if with_bias:
            m.update({
                'bqr': np.ascontiguousarray(
                    np.asarray(bq, f32)[None, cs:ce] * SCALE),
                'bkr': np.ascontiguousarray(np.asarray(bk, f32)[None, cs:ce]),
                'bvr': np.ascontiguousarray(np.asarray(bv, f32)[None, cs:ce]),
                'boqr': _pad_obs(np.asarray(boq, f32)[None, os_:oe] * OBS_SCALE),
                'bokr': _pad_obs(np.asarray(bok, f32)[None, os_:oe]),
                'onesd': np.ones((1, 512), f32),
            })
        in_maps.append(m)
    return in_maps


def kernel(**inputs):
    with_bias = any(
        np.any(np.asarray(inputs[k])) for k in ('bq', 'bk', 'bv', 'boq', 'bok'))
    nc = _get_nc(with_bias)
    in_maps = _host_prep(**inputs, with_bias=with_bias)
    res = run_bass_kernel_spmd(nc, in_maps, core_ids=list(range(NCORES)))
    bo = np.asarray(inputs['bo'], np.float32)
    outf = np.zeros((B, N, D), np.float32)
    for c in range(NCORES):
        outf[c // 2] += res.results[c]['out']
    outf += bo[None, None, :]
    return outf.reshape(B, T, V, D)


# revision 12
# speedup vs baseline: 1.0247x; 1.0043x over previous
"""Trainium2 Bass kernel for ClinicalStateFormationOperator (v2, pipelined).

Full-input contract: kernel(**inputs) takes the complete (unsharded) numpy
inputs and returns the full [B, T, V, D] output. Work is sharded across 8
NeuronCores as (batch, head-group): core c handles batch c//2 and heads
(c%2)*4 .. (c%2)*4+3. Each core computes its 4 heads' attention and the
partial output projection; the host sums the two partial projections per
batch and adds the output bias.

Math identical to v1 (scores computed transposed with a packed K=128
contraction: content 64 | obs 16 | variable-bias one-hots 32 | time-bias
basis 16; E^T = exp(scores^T); [out^T; denom] = [v_h | ones]^T @ E^T;
divide; partial out-projection). 1/sqrt(hd) and 1/sqrt(od) are folded into
Wq/Woq host-side so all PSUM->SBUF moves are plain copies.

Scheduling (the point of v2): the score->exp->AV chain is software-
pipelined one group ahead (PE emits scores for group i+1 before the AV of
group i), the Activation engine runs the exps plus the early-phase copies
that fit its pre-steady-state idle windows, all other PSUM->SBUF copies
run on DVE (gpsimd cannot access PSUM), and the projection matmuls are
emitted as deadline-tagged fillers inside the attention loop. Per-engine
execution order is pinned to emission order with nosync dependency chains
(see _chain_cb) because the Tile scheduler's region tracker only gives the
first-emitted reader of a region an edge to its writer.

v is stored interleaved [128, KC, head, {v|ones}, 64] so each head's AV
stationary operand [v_h | ones] is one contiguous 128-wide block and each
key chunk needs only a single strided copy from the projection PSUM.

PSUM: 4 banks score double-buffer ([128,2,512] x2) + 2 banks attention-out
([128,512] x2) + 2 banks projections/y ([128,512] x2).
"""

import ml_dtypes
import numpy as np

import concourse.bass as bass
import concourse.mybir as mybir
import concourse.tile as tile
from concourse.bass_utils import run_bass_kernel_spmd
from concourse.instruction_name_ordered_set import InstructionNameOrderedSet

V = 32
T = 48
D = 512
H = 8
HD = D // H          # 64
OD = 16
B = 4
N = T * V            # 1536
HPC = 4              # heads per core
NCORES = 8
SCALE = 1.0 / np.sqrt(HD)
OBS_SCALE = 1.0 / np.sqrt(OD)

F32 = mybir.dt.float32
F32R = mybir.dt.float32r
BF16 = mybir.dt.bfloat16
EXP = mybir.ActivationFunctionType.Exp

# bf16 q/k/v projection inputs: halves the startup DMA (which gates the
# first ~25 groups) for ~1e-2 worst-case relative error, well under the
# 2e-2 gate. Score packs, biases and the out-projection stay float32r.
PROJ_BF16 = True
PDT = BF16 if PROJ_BF16 else F32R

KC = N // 128        # 12 key chunks of 128
QC = N // 512        # 3 query chunks of 512
GPS = 6              # score/exp/AV groups per (head, qchunk): 2 key chunks

# slot order: j outer, h inner (v1 order). All slots need all keys, so the
# hard-early deliveries are fhT + k-proj(m0) only; q/obs/apack columns for
# j1/j2 and the m=1 packs have 12+ groups of slack.
SLOTS = [(j, h) for j in range(QC) for h in range(HPC)]


def _split_waits(nc, max_waits=1):
    """Walrus in this container allows only one sync-wait slot per
    instruction; spill extra waits onto preceding same-engine NoOps."""
    def fix_bb(bb):
        changed = False
        new = []
        for inst in bb.instructions:
            si = inst.sync_info
            if si is not None and len(si.on_wait) > max_waits:
                waits = list(si.on_wait)
                for w in waits[:-max_waits]:
                    new.append(mybir.InstNoOp(
                        name=nc.get_next_instruction_name(),
                        engine=inst.engine, ins=[], outs=[],
                        sync_info=mybir.SyncInfo(on_wait=[w], on_update=[])))
                    changed = True
                si.on_wait = waits[-max_waits:]
            new.append(inst)
        if changed:
            bb.instructions = new
        for sub in getattr(bb, 'blocks', []) or []:
            fix_bb(sub)
    for f in nc.m.functions:
        for bb in f.blocks:
            fix_bb(bb)


def _build(with_bias=False):
    nc = bass.Bass()

    # The Tile scheduler reorders same-engine instructions while its region
    # tracker gives only the FIRST-emitted reader of a region an edge to the
    # writer; a later reader hoisted above that first reader can then run
    # before the data lands (observed: score matmuls reading uninitialized
    # pack rows). Chain every compute instruction to its same-engine
    # predecessor so the per-engine order is exactly emission order, which
    # restores the transitivity the tracker's dedup assumes.
    _chain_last = {}
    _chain_types = (mybir.InstMatmult, mybir.InstTensorCopy,
                    mybir.InstActivation, mybir.InstReciprocal,
                    mybir.InstTensorTensor)
    _chain_engines = (mybir.EngineType.PE, mybir.EngineType.DVE,
                      mybir.EngineType.Activation)
    def _chain_cb(inst):
        if inst.engine in _chain_engines and isinstance(inst, _chain_types):
            prev = _chain_last.get(inst.engine)
            if prev is not None:
                deps = InstructionNameOrderedSet()
                deps.add(prev.name)
                # nosync: same-engine ordering is free (in-order engines);
                # a sync dep here would cost a redundant semaphore wait
                inst.add_nosync_dependencies_from(deps)
            _chain_last[inst.engine] = inst
    nc._state.push_inst_callback(_chain_cb)

    # ---- per-core DRAM I/O (data differs per core, program is SPMD) ----
    fhT = nc.dram_tensor('fhT', [D, N], PDT, kind='ExternalInput')
    foT = nc.dram_tensor('foT', [2, N], F32R, kind='ExternalInput')
    wq = nc.dram_tensor('wq', [D, HPC * HD], PDT, kind='ExternalInput')
    wk = nc.dram_tensor('wk', [D, HPC * HD], PDT, kind='ExternalInput')
    wv = nc.dram_tensor('wv', [D, HPC * HD], PDT, kind='ExternalInput')
    woq = nc.dram_tensor('woq', [2, 128], F32R, kind='ExternalInput')
    wok = nc.dram_tensor('wok', [2, 128], F32R, kind='ExternalInput')
    wo = nc.dram_tensor('wo', [2, 128, D], F32R, kind='ExternalInput')
    # score-bias expansion tables (host-gathered from variable_bias / rtb)
    vones = nc.dram_tensor('vones', [1, 256], F32R, kind='ExternalInput')
    kstat = nc.dram_tensor('kstat', [32, N], F32R, kind='ExternalInput')
    qstat = nc.dram_tensor('qstat', [HPC, 48, N], F32R, kind='ExternalInput')
    apack = nc.dram_tensor('apack', [HPC, QC, 16, N], F32R,
                           kind='ExternalInput')
    if with_bias:
        bqr = nc.dram_tensor('bqr', [1, HPC * HD], F32R, kind='ExternalInput')
        bkr = nc.dram_tensor('bkr', [1, HPC * HD], F32R, kind='ExternalInput')
        bvr = nc.dram_tensor('bvr', [1, HPC * HD], F32R, kind='ExternalInput')
        boqr = nc.dram_tensor('boqr', [1, 128], F32R, kind='ExternalInput')
        bokr = nc.dram_tensor('bokr', [1, 128], F32R, kind='ExternalInput')
        onesd = nc.dram_tensor('onesd', [1, 512], F32R, kind='ExternalInput')
    out = nc.dram_tensor('out', [N, D], F32, kind='ExternalOutput')

    with tile.TileContext(nc) as tc:
        with tc.tile_pool(name='cst', bufs=1) as cst, \
             tc.tile_pool(name='big', bufs=1) as big, \
             tc.tile_pool(name='work', bufs=3) as work, \
             tc.tile_pool(name='et', bufs=3) as etp, \
             tc.tile_pool(name='ps2', bufs=2, space='PSUM') as ps2, \
             tc.tile_pool(name='po', bufs=2, space='PSUM') as po, \
             tc.tile_pool(name='pp', bufs=2, space='PSUM') as pp:

            # ---- SBUF tiles ----
            t_foT = cst.tile([2, N], F32R)
            t_woq = cst.tile([2, 128], F32R)
            t_wok = cst.tile([2, 128], F32R)
            t_wk = cst.tile([128, 4, HPC * HD], PDT)
            t_wq = cst.tile([128, 4, HPC * HD], PDT)
            t_wv = cst.tile([128, 4, HPC * HD], PDT)
            t_wo = cst.tile([128, 2, D], F32R)
            t_fhT = big.tile([128, 4, N], PDT)
            t_qp = [big.tile([128, N], F32R, tag=f'qp{hh}', name=f'qp{hh}')
                    for hh in range(HPC)]
            t_kp = [big.tile([128, N], F32R, tag=f'kp{hh}', name=f'kp{hh}')
                    for hh in range(HPC)]
            # v interleaved with denominator-ones per head
            t_vx = big.tile([128, KC, HPC, 2, 64], F32R, tag='vx', name='vx')
            t_ot = [big.tile([128, N], F32R, tag=f'ot{p}', name=f'ot{p}')
                    for p in range(2)]

            # ---- DMA order: tracks consumption order ----
            t_v1 = cst.tile([1, 256], F32R)
            nc.sync.dma_start(t_v1[:], vones[:])
            nc.sync.dma_start(t_foT[:], foT[:])
            nc.sync.dma_start(t_woq[:], woq[:])
            nc.sync.dma_start(t_wok[:], wok[:])
            fhT_r = fhT[:].rearrange('(o p) n -> p o n', p=128)
            wk_r = wk[:].rearrange('(o p) n -> p o n', p=128)
            wq_r = wq[:].rearrange('(o p) n -> p o n', p=128)
            nc.sync.dma_start(t_fhT[:], fhT_r[:])
            nc.sync.dma_start(t_wk[:], wk_r[:])
            nc.sync.dma_start(t_wq[:], wq_r[:])
            nc.sync.dma_start(t_wv[:], wv[:].rearrange('(o p) n -> p o n', p=128))
            nc.sync.dma_start(t_kp[0][80:112, :], kstat[:])
            nc.sync.dma_start(t_qp[0][80:128, 0:512], qstat[0][:, 0:512])
            nc.sync.dma_start(t_kp[0][112:128, :], apack[0, 0])
            nc.sync.dma_start(t_kp[1][80:112, :], kstat[:])
            nc.sync.dma_start(t_qp[1][80:128, 0:512], qstat[1][:, 0:512])
            nc.sync.dma_start(t_kp[1][112:128, :], apack[1, 0])
            for hh in range(2, HPC):
                nc.sync.dma_start(t_kp[hh][80:112, :], kstat[:])
                nc.sync.dma_start(t_qp[hh][80:128, 0:512],
                                  qstat[hh][:, 0:512])
                nc.sync.dma_start(t_kp[hh][112:128, :], apack[hh, 0])
            for hh in range(HPC):
                nc.sync.dma_start(t_qp[hh][80:128, 512:N],
                                  qstat[hh][:, 512:N])
            nc.sync.dma_start(t_wo[:], wo[:].rearrange('o p n -> p o n'))
            if with_bias:
                t_bq = cst.tile([1, HPC * HD], F32R)
                nc.sync.dma_start(t_bq[:], bqr[:])
                t_bk = cst.tile([1, HPC * HD], F32R)
                nc.sync.dma_start(t_bk[:], bkr[:])
                t_bv = cst.tile([1, HPC * HD], F32R)
                nc.sync.dma_start(t_bv[:], bvr[:])
                t_boq = cst.tile([1, 128], F32R)
                nc.sync.dma_start(t_boq[:], boqr[:])
                t_bok = cst.tile([1, 128], F32R)
                nc.sync.dma_start(t_bok[:], bokr[:])
                t_ones = cst.tile([1, 512], F32R)
                nc.sync.dma_start(t_ones[:], onesd[:])

            # ---- projection emitters (used pre-loop and as fillers) ----
            # gpsimd cannot touch PSUM on real HW (BIR verifier), so every
            # PSUM->SBUF copy is DVE, except the j0 obs copies which ride
            # the Activation engine's idle window before its first exp.
            # NOTE the asymmetry: oq->qp columns are QUERY-chunked (needed
            # only by slots of that j), but ok->kp columns are KEY-chunked
            # (every slot reads all of them from group 2 on), so the ok
            # sides must all land before the first slot's later groups.
            def emit_obs_side(j, which, act_eng=False, pool=None):
                w_t = t_woq if which == 'oq' else t_wok
                pack = t_qp if which == 'oq' else t_kp
                p_o = (pool or pp).tile([128, 512],
                                        F32, tag='sc' if pool else 'pp',
                                        name='p_o')
                nc.tensor.matmul(p_o[:], w_t[:],
                                 t_foT[:, j * 512:(j + 1) * 512],
                                 start=True, stop=(not with_bias))
                if with_bias:
                    bt = t_boq if which == 'oq' else t_bok
                    nc.tensor.matmul(p_o[:], bt[:], t_ones[:],
                                     start=False, stop=True)
                for hh in range(HPC):
                    dst = pack[hh][64:80, j * 512:(j + 1) * 512]
                    srcp = p_o[hh * 32:hh * 32 + OD, :]
                    on_act = act_eng is True or (act_eng == 'split' and hh < 2)
                    if on_act:
                        nc.scalar.copy(dst, srcp)
                    else:
                        nc.vector.tensor_copy(dst, srcp)

            def emit_qk(m, j, which):
                # q or k projection for head pair m, query chunk j
                w_t = t_wq if which == 'q' else t_wk
                pack = t_qp if which == 'q' else t_kp
                p_qt = pp.tile([128, 512], F32, tag='pp', name='p_qt')
                for kk in range(4):
                    nc.tensor.matmul(
                        p_qt[:], w_t[:, kk, m * 128:(m + 1) * 128],
                        t_fhT[:, kk, j * 512:(j + 1) * 512],
                        start=(kk == 0), stop=(not with_bias and kk == 3))
                if with_bias:
                    bt = t_bq if which == 'q' else t_bk
                    nc.tensor.matmul(p_qt[:], bt[:, m * 128:(m + 1) * 128],
                                     t_ones[:], start=False, stop=True)
                for s in range(2):
                    hh = 2 * m + s
                    nc.vector.tensor_copy(
                        pack[hh][0:64, j * 512:(j + 1) * 512],
                        p_qt[s * 64:(s + 1) * 64, :])

            def emit_v(kc, act_eng=False):
                # v projection for key chunk kc, plus the denominator-ones
                # block (written by a K=1 ones matmul so every byte of the
                # copy source is dependency-tracked); one strided copy
                # interleaves [v_h | ones] per head into t_vx
                p_v = pp.tile([128, 2, HPC * 64], F32, tag='pp', name='p_v')
                for kk in range(4):
                    nc.tensor.matmul(p_v[:, 0, :],
                                     t_fhT[:, kk, kc * 128:(kc + 1) * 128],
                                     t_wv[:, kk, :], start=(kk == 0),
                                     stop=(not with_bias and kk == 3))
                if with_bias:
                    nc.tensor.matmul(p_v[:, 0, :], t_ones[:, 0:128],
                                     t_bv[:], start=False, stop=True)
                nc.tensor.matmul(p_v[:, 1, :], t_v1[:, 0:128], t_v1[:],
                                 start=True, stop=True)
                src_ap = p_v[:].rearrange('p s (h c) -> p h s c', h=HPC)
                if act_eng:
                    nc.scalar.copy(t_vx[:, kc, :, :, :], src_ap)
                else:
                    nc.vector.tensor_copy(t_vx[:, kc, :, :, :], src_ap)

            def emit_y_qq(j, qq, t_y):
                # one 128-row block of the partial out-projection for query
                # chunk j; the last chunk's copies ride the Activation
                # engine (idle after its final exp) to shorten the tail
                qc = 4 * j + qq
                p_y = pp.tile([128, D], F32, tag='pp', name='p_y')
                for p in range(2):
                    nc.tensor.matmul(p_y[:],
                                     t_ot[p][:, qc * 128:(qc + 1) * 128],
                                     t_wo[:, p, :], start=(p == 0),
                                     stop=(p == 1))
                if j == QC - 1:
                    nc.scalar.copy(t_y[:, qq, :], p_y[:])
                else:
                    nc.vector.tensor_copy(t_y[:, qq, :], p_y[:])
                nc.sync.dma_start(out[qc * 128:(qc + 1) * 128, :],
                                  t_y[:, qq, :])

            # ---- pre-loop: only what slot 0 group 0 needs early ----
            emit_obs_side(0, 'oq', act_eng=True)
            emit_obs_side(0, 'ok', act_eng=True)
            emit_obs_side(1, 'ok')
            emit_qk(0, 0, 'k')
            emit_qk(0, 0, 'q')

            # fillers: (deadline_group_idx, closure), deadline-sorted, in
            # true readiness order. ~3 groups of slack for the psum->copy
            # chain latency ahead of each consumer.
            fillers = [
                (1, lambda: emit_v(0, act_eng=True)),
                (1, lambda: emit_v(1, act_eng=True)),
                (2, lambda: emit_qk(0, 1, 'k')),       # keys 512:1024 at g2
                (2, lambda: emit_v(2, act_eng=True)),
                (2, lambda: emit_v(3, act_eng=True)),
                (3, lambda: emit_qk(0, 2, 'k')),       # keys 1024:1536 at g4
                (3, lambda: emit_v(4, act_eng=True)),
                (3, lambda: emit_v(5, act_eng=True)),
                (3, lambda: emit_obs_side(2, 'ok', act_eng='split')),
                (4, lambda: emit_v(6)), (4, lambda: emit_v(7)),
                (5, lambda: emit_v(8)), (5, lambda: emit_v(9)),
                (6, lambda: emit_v(10)), (6, lambda: emit_v(11)),
                (6, lambda: emit_qk(1, 0, 'k')),       # slot 2 = (j0, h2)
                (7, lambda: emit_qk(1, 0, 'q')),
                (8, lambda: emit_qk(1, 1, 'k')),
                (10, lambda: emit_qk(1, 2, 'k')),
                (14, lambda: emit_obs_side(1, 'oq')),  # slot 4 = (j1, h0)
                (15, lambda: emit_qk(0, 1, 'q')),
                (28, lambda: emit_qk(1, 1, 'q')),      # slot 6 = (j1, h2)
                (38, lambda: emit_obs_side(2, 'oq')),  # slot 8 = (j2, h0)
                (39, lambda: emit_qk(0, 2, 'q')),
                (52, lambda: emit_qk(1, 2, 'q')),      # slot 10 = (j2, h2)
            ]
            fi = 0

            # ---- main attention loop: score/exp/AV pipelined one group ----
            seq = [(si, g) for si in range(len(SLOTS)) for g in range(GPS)]
            n_groups = len(seq)
            pend = None          # (slot_idx, g, p_ot, t_et) awaiting AV

            def emit_scores(si, g):
                j, hh = SLOTS[si]
                p_s = ps2.tile([128, 2, 512], F32, tag='sc', name='p_s')
                mm = None
                for i2 in range(2):
                    kc = 2 * g + i2
                    mm = nc.tensor.matmul(
                        p_s[:, i2, :],
                        t_kp[hh][:, kc * 128:(kc + 1) * 128],
                        t_qp[hh][:, j * 512:(j + 1) * 512],
                        start=True, stop=True)
                t_et = etp.tile([128, 2, 512], F32R, tag='et', name='t_et')
                nc.scalar.activation(t_et[:], p_s[:], EXP)
                return t_et, mm

            def emit_av(si, g, p_ot, t_et):
                j, hh = SLOTS[si]
                for i2 in range(2):
                    kc = 2 * g + i2
                    nc.tensor.matmul(
                        p_ot[:],
                        t_vx[:, kc, hh].rearrange('p a c -> p (a c)'),
                        t_et[:, i2, :],
                        start=(kc == 0), stop=(kc == KC - 1))

            pend_y = []

            def finish_slot(si, p_ot):
                j, hh = SLOTS[si]
                t_rec = work.tile([64, 512], F32, tag='rec', name='t_rec')
                nc.vector.reciprocal(t_rec[:], p_ot[64:128, :])
                nc.vector.tensor_mul(
                    t_ot[hh // 2][(hh % 2) * 64:(hh % 2) * 64 + 64,
                                  j * 512:(j + 1) * 512],
                    p_ot[0:64, :], t_rec[:])
                if hh == HPC - 1:
                    # spread the 4 out-projection blocks across the next
                    # groups so they don't wall off the next slot's scores
                    t_y = work.tile([128, 4, D], F32, tag='y', name='t_y')
                    for qq in range(4):
                        pend_y.append((j, qq, t_y))

            p_ot_by_slot = {}
            y_skip = 0
            for idx in range(n_groups):
                si, g = seq[idx]
                while fi < len(fillers) and fillers[fi][0] <= idx:
                    fillers[fi][1]()
                    fi += 1
                if g == 0:
                    p_ot_by_slot[si] = po.tile([128, 512], F32, tag='po',
                                               name='p_ot')
                p_ot_cur = p_ot_by_slot[si]
                t_et, last_mm = emit_scores(si, g)
                if g == GPS - 1:
                    # next j for this head reuses kp rows 112:128. Tile does
                    # NOT track the DMA-write-after-PE-read hazard here (the
                    # reload only gets DMA-queue-ring waits), so add the WAR
                    # edge on this slot's last score matmul explicitly: the
                    # PE stream is in-order, so that one edge covers all 12
                    # score reads of the old rows.
                    j, hh = SLOTS[si]
                    if j + 1 < QC:
                        # issue from the otherwise-idle gpsimd engine: the
                        # WAR wait must not block SP's DMA stream (it would
                        # hold up y-output DMAs and cross-couple queues)
                        dma = nc.gpsimd.dma_start(t_kp[hh][112:128, :],
                                                  apack[hh, j + 1])
                        deps = InstructionNameOrderedSet()
                        deps.add(last_mm.ins.name)
                        dma.ins.add_sync_dependencies_from(deps)
                if pend is not None:
                    emit_av(*pend)
                    if pend[1] == GPS - 1:
                        finish_slot(pend[0], pend[2])
                        if pend_y:
                            # let the divide (recip+mul) clear the DVE chain
                            # before the first y-projection reads t_ot, and
                            # spread the blocks so scores aren't walled off
                            y_skip = 2
                if pend_y:
                    if y_skip > 0:
                        y_skip -= 1
                    elif idx % 2 == 0:
                        emit_y_qq(*pend_y.pop(0))
                pend = (si, g, p_ot_cur, t_et)
            emit_av(*pend)
            # tail: split the last divide into column halves so the first
            # out-projection blocks start as soon as their columns are ready
            lsi, lp_ot = pend[0], pend[2]
            lj, lhh = SLOTS[lsi]
            t_y_last = work.tile([128, 4, D], F32, tag='y', name='t_y')
            for half in range(2):
                cs0, cs1 = half * 256, half * 256 + 256
                t_rec = work.tile([64, 256], F32, tag='rec', name='t_rec')
                nc.vector.reciprocal(t_rec[:], lp_ot[64:128, cs0:cs1])
                nc.vector.tensor_mul(
                    t_ot[lhh // 2][(lhh % 2) * 64:(lhh % 2) * 64 + 64,
                                   lj * 512 + cs0:lj * 512 + cs1],
                    lp_ot[0:64, cs0:cs1], t_rec[:])
                for qq in (2 * half, 2 * half + 1):
                    emit_y_qq(lj, qq, t_y_last)
            while pend_y:
                emit_y_qq(*pend_y.pop(0))

    nc._state.remove_inst_callback(_chain_cb)
    _split_waits(nc)
    return nc


_NC_CACHE = {}


def _get_nc(with_bias=False):
    if with_bias not in _NC_CACHE:
        _NC_CACHE[with_bias] = _build(with_bias)
    return _NC_CACHE[with_bias]


def _pad_obs(a):
    # lay each head's 16 obs channels at a 32-column boundary (PSUM reads
    # must start at 32-partition-aligned offsets)
    out = np.zeros((a.shape[0], 128), np.float32)
    for hh in range(HPC):
        out[:, hh * 32:hh * 32 + OD] = a[:, hh * OD:(hh + 1) * OD]
    return out


def _host_prep(h, observation_state, Wq, bq, Wk, bk, Wv, bv, Wo, bo,
               Woq, boq, Wok, bok, variable_bias, relative_time_bias,
               with_bias=False):
    f32 = np.float32
    h = np.asarray(h, f32)
    obs = np.asarray(observation_state, f32)
    Kidx = np.arange(N)
    tK = Kidx // V                                     # time bin of each token
    kstat = (Kidx[None, :] % V == np.arange(V)[:, None]).astype(f32)
    bq16 = ((Kidx[None, :] // V) % 16 == np.arange(16)[:, None]).astype(f32)

    in_maps = []
    for c in range(NCORES):
        b, hg = divmod(c, 2)
        h0 = hg * HPC
        cs, ce = h0 * HD, (h0 + HPC) * HD
        os_, oe = h0 * OD, (h0 + HPC) * OD
        qstat = np.empty((HPC, 48, N), f32)
        ap = np.empty((HPC, QC, 16, N), f32)
        for hh in range(HPC):
            head = h0 + hh
            vb = np.asarray(variable_bias[head], f32)
            rtb = np.asarray(relative_time_bias[head], f32)
            qstat[hh, :V] = vb[Kidx % V, :].T          # VB_h[Q%32, j]
            qstat[hh, V:] = bq16
            for j in range(QC):
                # A_hj[s, K] = rtb[16j + s - K//32 + 47]
                idx = 16 * j + np.arange(16)[:, None] - tK[None, :] + (T - 1)
                ap[hh, j] = rtb[idx]
        pdt = ml_dtypes.bfloat16 if PROJ_BF16 else f32
        m = {
            'fhT': np.ascontiguousarray(h[b].reshape(N, D).T).astype(pdt),
            'foT': np.ascontiguousarray(obs[b].reshape(N, 2).T),
            # content / obs scales folded into the q-side weights
            'wq': np.ascontiguousarray(
                np.asarray(Wq, f32)[:, cs:ce] * SCALE).astype(pdt),
            'wk': np.ascontiguousarray(
                np.asarray(Wk, f32)[:, cs:ce]).astype(pdt),
            'wv': np.ascontiguousarray(
                np.asarray(Wv, f32)[:, cs:ce]).astype(pdt),
            'woq': _pad_obs(np.asarray(Woq, f32)[:, os_:oe] * OBS_SCALE),
            'wok': _pad_obs(np.asarray(Wok, f32)[:, os_:oe]),
            'wo': np.ascontiguousarray(
                np.asarray(Wo, f32)[cs:ce, :].reshape(2, 128, D)),
            'vones': np.ones((1, 256), f32),
            'kstat': kstat,
            'qstat': qstat,
            'apack': ap,
        }
        if with_bias:
            m.update({
                'bqr': np.ascontiguousarray(
                    np.asarray(bq, f32)[None, cs:ce] * SCALE),
                'bkr': np.ascontiguousarray(np.asarray(bk, f32)[None, cs:ce]),
                'bvr': np.ascontiguousarray(np.asarray(bv, f32)[None, cs:ce]),
                'boqr': _pad_obs(np.asarray(boq, f32)[None, os_:oe] * OBS_SCALE),
                'bokr': _pad_obs(np.asarray(bok, f32)[None, os_:oe]),
                'onesd': np.ones((1, 512), f32),
            })
        in_maps.append(m)
    return in_maps


def kernel(**inputs):
    with_bias = any(
        np.any(np.asarray(inputs[k])) for k in ('bq', 'bk', 'bv', 'boq', 'bok'))
    nc = _get_nc(with_bias)
    in_maps = _host_prep(**inputs, with_bias=with_bias)
    res = run_bass_kernel_spmd(nc, in_maps, core_ids=list(range(NCORES)))
    bo = np.asarray(inputs['bo'], np.float32)
    outf = np.zeros((B, N, D), np.float32)
    for c in range(NCORES):
        outf[c // 2] += res.results[c]['out']
    outf += bo[None, None, :]
    return outf.reshape(B, T, V, D)


# revision 13
# speedup vs baseline: 1.0284x; 1.0036x over previous
"""Trainium2 Bass kernel for ClinicalStateFormationOperator (v2, pipelined).

Full-input contract: kernel(**inputs) takes the complete (unsharded) numpy
inputs and returns the full [B, T, V, D] output. Work is sharded across 8
NeuronCores as (batch, head-group): core c handles batch c//2 and heads
(c%2)*4 .. (c%2)*4+3. Each core computes its 4 heads' attention and the
partial output projection; the host sums the two partial projections per
batch and adds the output bias.

Math identical to v1 (scores computed transposed with a packed K=128
contraction: content 64 | obs 16 | variable-bias one-hots 32 | time-bias
basis 16; E^T = exp(scores^T); [out^T; denom] = [v_h | ones]^T @ E^T;
divide; partial out-projection). 1/sqrt(hd) and 1/sqrt(od) are folded into
Wq/Woq host-side so all PSUM->SBUF moves are plain copies.

Scheduling (the point of v2): the score->exp->AV chain is software-
pipelined one group ahead (PE emits scores for group i+1 before the AV of
group i), the Activation engine runs the exps plus the early-phase copies
that fit its pre-steady-state idle windows, all other PSUM->SBUF copies
run on DVE (gpsimd cannot access PSUM), and the projection matmuls are
emitted as deadline-tagged fillers inside the attention loop. Per-engine
execution order is pinned to emission order with nosync dependency chains
(see _chain_cb) because the Tile scheduler's region tracker only gives the
first-emitted reader of a region an edge to its writer.

v is stored interleaved [128, KC, head, {v|ones}, 64] so each head's AV
stationary operand [v_h | ones] is one contiguous 128-wide block and each
key chunk needs only a single strided copy from the projection PSUM.

PSUM: 4 banks score double-buffer ([128,2,512] x2) + 2 banks attention-out
([128,512] x2) + 2 banks projections/y ([128,512] x2).
"""

import ml_dtypes
import numpy as np

import concourse.bass as bass
import concourse.mybir as mybir
import concourse.tile as tile
from concourse.bass_utils import run_bass_kernel_spmd
from concourse.instruction_name_ordered_set import InstructionNameOrderedSet

V = 32
T = 48
D = 512
H = 8
HD = D // H          # 64
OD = 16
B = 4
N = T * V            # 1536
HPC = 4              # heads per core
NCORES = 8
SCALE = 1.0 / np.sqrt(HD)
OBS_SCALE = 1.0 / np.sqrt(OD)

F32 = mybir.dt.float32
F32R = mybir.dt.float32r
BF16 = mybir.dt.bfloat16
EXP = mybir.ActivationFunctionType.Exp

# bf16 q/k/v projection inputs: halves the startup DMA (which gates the
# first ~25 groups) for ~1e-2 worst-case relative error, well under the
# 2e-2 gate. Score packs, biases and the out-projection stay float32r.
PROJ_BF16 = True
PDT = BF16 if PROJ_BF16 else F32R

KC = N // 128        # 12 key chunks of 128
QC = N // 512        # 3 query chunks of 512
GPS = 6              # score/exp/AV groups per (head, qchunk): 2 key chunks

# slot order: j outer, h inner (v1 order). All slots need all keys, so the
# hard-early deliveries are fhT + k-proj(m0) only; q/obs/apack columns for
# j1/j2 and the m=1 packs have 12+ groups of slack.
SLOTS = [(j, h) for j in range(QC) for h in range(HPC)]


def _split_waits(nc, max_waits=1):
    """Walrus in this container allows only one sync-wait slot per
    instruction; spill extra waits onto preceding same-engine NoOps."""
    def fix_bb(bb):
        changed = False
        new = []
        for inst in bb.instructions:
            si = inst.sync_info
            if si is not None and len(si.on_wait) > max_waits:
                waits = list(si.on_wait)
                for w in waits[:-max_waits]:
                    new.append(mybir.InstNoOp(
                        name=nc.get_next_instruction_name(),
                        engine=inst.engine, ins=[], outs=[],
                        sync_info=mybir.SyncInfo(on_wait=[w], on_update=[])))
                    changed = True
                si.on_wait = waits[-max_waits:]
            new.append(inst)
        if changed:
            bb.instructions = new
        for sub in getattr(bb, 'blocks', []) or []:
            fix_bb(sub)
    for f in nc.m.functions:
        for bb in f.blocks:
            fix_bb(bb)


def _build(with_bias=False):
    nc = bass.Bass()

    # The Tile scheduler reorders same-engine instructions while its region
    # tracker gives only the FIRST-emitted reader of a region an edge to the
    # writer; a later reader hoisted above that first reader can then run
    # before the data lands (observed: score matmuls reading uninitialized
    # pack rows). Chain every compute instruction to its same-engine
    # predecessor so the per-engine order is exactly emission order, which
    # restores the transitivity the tracker's dedup assumes.
    _chain_last = {}
    _chain_types = (mybir.InstMatmult, mybir.InstTensorCopy,
                    mybir.InstActivation, mybir.InstReciprocal,
                    mybir.InstTensorTensor)
    _chain_engines = (mybir.EngineType.PE, mybir.EngineType.DVE,
                      mybir.EngineType.Activation)
    def _chain_cb(inst):
        if inst.engine in _chain_engines and isinstance(inst, _chain_types):
            prev = _chain_last.get(inst.engine)
            if prev is not None:
                deps = InstructionNameOrderedSet()
                deps.add(prev.name)
                # nosync: same-engine ordering is free (in-order engines);
                # a sync dep here would cost a redundant semaphore wait
                inst.add_nosync_dependencies_from(deps)
            _chain_last[inst.engine] = inst
    nc._state.push_inst_callback(_chain_cb)

    # ---- per-core DRAM I/O (data differs per core, program is SPMD) ----
    fhT = nc.dram_tensor('fhT', [D, N], PDT, kind='ExternalInput')
    foT = nc.dram_tensor('foT', [2, N], F32R, kind='ExternalInput')
    wq = nc.dram_tensor('wq', [D, HPC * HD], PDT, kind='ExternalInput')
    wk = nc.dram_tensor('wk', [D, HPC * HD], PDT, kind='ExternalInput')
    wv = nc.dram_tensor('wv', [D, HPC * HD], PDT, kind='ExternalInput')
    woq = nc.dram_tensor('woq', [2, 128], F32R, kind='ExternalInput')
    wok = nc.dram_tensor('wok', [2, 128], F32R, kind='ExternalInput')
    wo = nc.dram_tensor('wo', [2, 128, D], F32R, kind='ExternalInput')
    # score-bias expansion tables (host-gathered from variable_bias / rtb)
    vones = nc.dram_tensor('vones', [1, 256], F32R, kind='ExternalInput')
    kstat = nc.dram_tensor('kstat', [32, N], F32R, kind='ExternalInput')
    qstat = nc.dram_tensor('qstat', [HPC, 48, N], F32R, kind='ExternalInput')
    apack = nc.dram_tensor('apack', [HPC, QC, 16, N], F32R,
                           kind='ExternalInput')
    if with_bias:
        bqr = nc.dram_tensor('bqr', [1, HPC * HD], F32R, kind='ExternalInput')
        bkr = nc.dram_tensor('bkr', [1, HPC * HD], F32R, kind='ExternalInput')
        bvr = nc.dram_tensor('bvr', [1, HPC * HD], F32R, kind='ExternalInput')
        boqr = nc.dram_tensor('boqr', [1, 128], F32R, kind='ExternalInput')
        bokr = nc.dram_tensor('bokr', [1, 128], F32R, kind='ExternalInput')
        onesd = nc.dram_tensor('onesd', [1, 512], F32R, kind='ExternalInput')
    # bf16 output: halves the out-DMA transfers on the drain tail; the host
    # upconverts before summing the two partial projections per batch
    out = nc.dram_tensor('out', [N, D], BF16, kind='ExternalOutput')

    with tile.TileContext(nc) as tc:
        with tc.tile_pool(name='cst', bufs=1) as cst, \
             tc.tile_pool(name='big', bufs=1) as big, \
             tc.tile_pool(name='work', bufs=3) as work, \
             tc.tile_pool(name='et', bufs=3) as etp, \
             tc.tile_pool(name='ps2', bufs=2, space='PSUM') as ps2, \
             tc.tile_pool(name='po', bufs=2, space='PSUM') as po, \
             tc.tile_pool(name='pp', bufs=2, space='PSUM') as pp:

            # ---- SBUF tiles ----
            t_foT = cst.tile([2, N], F32R)
            t_woq = cst.tile([2, 128], F32R)
            t_wok = cst.tile([2, 128], F32R)
            t_wk = cst.tile([128, 4, HPC * HD], PDT)
            t_wq = cst.tile([128, 4, HPC * HD], PDT)
            t_wv = cst.tile([128, 4, HPC * HD], PDT)
            t_wo = cst.tile([128, 2, D], F32R)
            t_fhT = big.tile([128, 4, N], PDT)
            t_qp = [big.tile([128, N], F32R, tag=f'qp{hh}', name=f'qp{hh}')
                    for hh in range(HPC)]
            t_kp = [big.tile([128, N], F32R, tag=f'kp{hh}', name=f'kp{hh}')
                    for hh in range(HPC)]
            # v interleaved with denominator-ones per head
            t_vx = big.tile([128, KC, HPC, 2, 64], F32R, tag='vx', name='vx')
            t_ot = [big.tile([128, N], F32R, tag=f'ot{p}', name=f'ot{p}')
                    for p in range(2)]

            # ---- DMA order: tracks consumption order ----
            t_v1 = cst.tile([1, 256], F32R)
            nc.sync.dma_start(t_v1[:], vones[:])
            nc.sync.dma_start(t_foT[:], foT[:])
            nc.sync.dma_start(t_woq[:], woq[:])
            nc.sync.dma_start(t_wok[:], wok[:])
            fhT_r = fhT[:].rearrange('(o p) n -> p o n', p=128)
            wk_r = wk[:].rearrange('(o p) n -> p o n', p=128)
            wq_r = wq[:].rearrange('(o p) n -> p o n', p=128)
            nc.sync.dma_start(t_fhT[:], fhT_r[:])
            nc.sync.dma_start(t_wk[:], wk_r[:])
            nc.sync.dma_start(t_wq[:], wq_r[:])
            nc.sync.dma_start(t_wv[:], wv[:].rearrange('(o p) n -> p o n', p=128))
            nc.sync.dma_start(t_kp[0][80:112, :], kstat[:])
            nc.sync.dma_start(t_qp[0][80:128, 0:512], qstat[0][:, 0:512])
            nc.sync.dma_start(t_kp[0][112:128, :], apack[0, 0])
            nc.sync.dma_start(t_kp[1][80:112, :], kstat[:])
            nc.sync.dma_start(t_qp[1][80:128, 0:512], qstat[1][:, 0:512])
            nc.sync.dma_start(t_kp[1][112:128, :], apack[1, 0])
            for hh in range(2, HPC):
                nc.sync.dma_start(t_kp[hh][80:112, :], kstat[:])
                nc.sync.dma_start(t_qp[hh][80:128, 0:512],
                                  qstat[hh][:, 0:512])
                nc.sync.dma_start(t_kp[hh][112:128, :], apack[hh, 0])
            for hh in range(HPC):
                nc.sync.dma_start(t_qp[hh][80:128, 512:N],
                                  qstat[hh][:, 512:N])
            nc.sync.dma_start(t_wo[:], wo[:].rearrange('o p n -> p o n'))
            if with_bias:
                t_bq = cst.tile([1, HPC * HD], F32R)
                nc.sync.dma_start(t_bq[:], bqr[:])
                t_bk = cst.tile([1, HPC * HD], F32R)
                nc.sync.dma_start(t_bk[:], bkr[:])
                t_bv = cst.tile([1, HPC * HD], F32R)
                nc.sync.dma_start(t_bv[:], bvr[:])
                t_boq = cst.tile([1, 128], F32R)
                nc.sync.dma_start(t_boq[:], boqr[:])
                t_bok = cst.tile([1, 128], F32R)
                nc.sync.dma_start(t_bok[:], bokr[:])
                t_ones = cst.tile([1, 512], F32R)
                nc.sync.dma_start(t_ones[:], onesd[:])

            # ---- projection emitters (used pre-loop and as fillers) ----
            # gpsimd cannot touch PSUM on real HW (BIR verifier), so every
            # PSUM->SBUF copy is DVE, except the j0 obs copies which ride
            # the Activation engine's idle window before its first exp.
            # NOTE the asymmetry: oq->qp columns are QUERY-chunked (needed
            # only by slots of that j), but ok->kp columns are KEY-chunked
            # (every slot reads all of them from group 2 on), so the ok
            # sides must all land before the first slot's later groups.
            def emit_obs_side(j, which, act_eng=False, pool=None):
                w_t = t_woq if which == 'oq' else t_wok
                pack = t_qp if which == 'oq' else t_kp
                p_o = (pool or pp).tile([128, 512],
                                        F32, tag='sc' if pool else 'pp',
                                        name='p_o')
                nc.tensor.matmul(p_o[:], w_t[:],
                                 t_foT[:, j * 512:(j + 1) * 512],
                                 start=True, stop=(not with_bias))
                if with_bias:
                    bt = t_boq if which == 'oq' else t_bok
                    nc.tensor.matmul(p_o[:], bt[:], t_ones[:],
                                     start=False, stop=True)
                for hh in range(HPC):
                    dst = pack[hh][64:80, j * 512:(j + 1) * 512]
                    srcp = p_o[hh * 32:hh * 32 + OD, :]
                    on_act = act_eng is True or (act_eng == 'split' and hh < 2)
                    if on_act:
                        nc.scalar.copy(dst, srcp)
                    else:
                        nc.vector.tensor_copy(dst, srcp)

            def emit_qk(m, j, which):
                # q or k projection for head pair m, query chunk j
                w_t = t_wq if which == 'q' else t_wk
                pack = t_qp if which == 'q' else t_kp
                p_qt = pp.tile([128, 512], F32, tag='pp', name='p_qt')
                for kk in range(4):
                    nc.tensor.matmul(
                        p_qt[:], w_t[:, kk, m * 128:(m + 1) * 128],
                        t_fhT[:, kk, j * 512:(j + 1) * 512],
                        start=(kk == 0), stop=(not with_bias and kk == 3))
                if with_bias:
                    bt = t_bq if which == 'q' else t_bk
                    nc.tensor.matmul(p_qt[:], bt[:, m * 128:(m + 1) * 128],
                                     t_ones[:], start=False, stop=True)
                for s in range(2):
                    hh = 2 * m + s
                    nc.vector.tensor_copy(
                        pack[hh][0:64, j * 512:(j + 1) * 512],
                        p_qt[s * 64:(s + 1) * 64, :])

            def emit_v(kc, act_eng=False):
                # v projection for key chunk kc, plus the denominator-ones
                # block (written by a K=1 ones matmul so every byte of the
                # copy source is dependency-tracked); one strided copy
                # interleaves [v_h | ones] per head into t_vx
                p_v = pp.tile([128, 2, HPC * 64], F32, tag='pp', name='p_v')
                for kk in range(4):
                    nc.tensor.matmul(p_v[:, 0, :],
                                     t_fhT[:, kk, kc * 128:(kc + 1) * 128],
                                     t_wv[:, kk, :], start=(kk == 0),
                                     stop=(not with_bias and kk == 3))
                if with_bias:
                    nc.tensor.matmul(p_v[:, 0, :], t_ones[:, 0:128],
                                     t_bv[:], start=False, stop=True)
                nc.tensor.matmul(p_v[:, 1, :], t_v1[:, 0:128], t_v1[:],
                                 start=True, stop=True)
                src_ap = p_v[:].rearrange('p s (h c) -> p h s c', h=HPC)
                if act_eng:
                    nc.scalar.copy(t_vx[:, kc, :, :, :], src_ap)
                else:
                    nc.vector.tensor_copy(t_vx[:, kc, :, :, :], src_ap)

            def emit_y_qq(j, qq, t_y):
                # one 128-row block of the partial out-projection for query
                # chunk j; the last chunk's copies ride the Activation
                # engine (idle after its final exp) to shorten the tail
                qc = 4 * j + qq
                p_y = pp.tile([128, D], F32, tag='pp', name='p_y')
                for p in range(2):
                    nc.tensor.matmul(p_y[:],
                                     t_ot[p][:, qc * 128:(qc + 1) * 128],
                                     t_wo[:, p, :], start=(p == 0),
                                     stop=(p == 1))
                if j == QC - 1:
                    nc.scalar.copy(t_y[:, qq, :], p_y[:])
                else:
                    nc.vector.tensor_copy(t_y[:, qq, :], p_y[:])
                nc.sync.dma_start(out[qc * 128:(qc + 1) * 128, :],
                                  t_y[:, qq, :])

            # ---- pre-loop: only what slot 0 group 0 needs early ----
            emit_obs_side(0, 'oq', act_eng=True)
            emit_obs_side(0, 'ok', act_eng=True)
            emit_obs_side(1, 'ok')
            emit_qk(0, 0, 'k')
            emit_qk(0, 0, 'q')

            # fillers: (deadline_group_idx, closure), deadline-sorted, in
            # true readiness order. ~3 groups of slack for the psum->copy
            # chain latency ahead of each consumer.
            fillers = [
                (1, lambda: emit_v(0, act_eng=True)),
                (1, lambda: emit_v(1, act_eng=True)),
                (2, lambda: emit_qk(0, 1, 'k')),       # keys 512:1024 at g2
                (2, lambda: emit_v(2, act_eng=True)),
                (2, lambda: emit_v(3, act_eng=True)),
                (3, lambda: emit_qk(0, 2, 'k')),       # keys 1024:1536 at g4
                (3, lambda: emit_v(4, act_eng=True)),
                (3, lambda: emit_v(5, act_eng=True)),
                (3, lambda: emit_obs_side(2, 'ok', act_eng='split')),
                (4, lambda: emit_v(6)), (4, lambda: emit_v(7)),
                (5, lambda: emit_v(8)), (5, lambda: emit_v(9)),
                (6, lambda: emit_v(10)), (6, lambda: emit_v(11)),
                (6, lambda: emit_qk(1, 0, 'k')),       # slot 2 = (j0, h2)
                (7, lambda: emit_qk(1, 0, 'q')),
                (8, lambda: emit_qk(1, 1, 'k')),
                (10, lambda: emit_qk(1, 2, 'k')),
                (14, lambda: emit_obs_side(1, 'oq')),  # slot 4 = (j1, h0)
                (15, lambda: emit_qk(0, 1, 'q')),
                (28, lambda: emit_qk(1, 1, 'q')),      # slot 6 = (j1, h2)
                (38, lambda: emit_obs_side(2, 'oq')),  # slot 8 = (j2, h0)
                (39, lambda: emit_qk(0, 2, 'q')),
                (52, lambda: emit_qk(1, 2, 'q')),      # slot 10 = (j2, h2)
            ]
            fi = 0

            # ---- main attention loop: score/exp/AV pipelined one group ----
            seq = [(si, g) for si in range(len(SLOTS)) for g in range(GPS)]
            n_groups = len(seq)
            pend = None          # (slot_idx, g, p_ot, t_et) awaiting AV

            def emit_scores(si, g):
                j, hh = SLOTS[si]
                p_s = ps2.tile([128, 2, 512], F32, tag='sc', name='p_s')
                mm = None
                for i2 in range(2):
                    kc = 2 * g + i2
                    mm = nc.tensor.matmul(
                        p_s[:, i2, :],
                        t_kp[hh][:, kc * 128:(kc + 1) * 128],
                        t_qp[hh][:, j * 512:(j + 1) * 512],
                        start=True, stop=True)
                t_et = etp.tile([128, 2, 512], F32R, tag='et', name='t_et')
                nc.scalar.activation(t_et[:], p_s[:], EXP)
                return t_et, mm

            def emit_av(si, g, p_ot, t_et):
                j, hh = SLOTS[si]
                for i2 in range(2):
                    kc = 2 * g + i2
                    nc.tensor.matmul(
                        p_ot[:],
                        t_vx[:, kc, hh].rearrange('p a c -> p (a c)'),
                        t_et[:, i2, :],
                        start=(kc == 0), stop=(kc == KC - 1))

            pend_y = []

            def finish_slot(si, p_ot):
                j, hh = SLOTS[si]
                t_rec = work.tile([64, 512], F32, tag='rec', name='t_rec')
                nc.vector.reciprocal(t_rec[:], p_ot[64:128, :])
                nc.vector.tensor_mul(
                    t_ot[hh // 2][(hh % 2) * 64:(hh % 2) * 64 + 64,
                                  j * 512:(j + 1) * 512],
                    p_ot[0:64, :], t_rec[:])
                if hh == HPC - 1:
                    # spread the 4 out-projection blocks across the next
                    # groups so they don't wall off the next slot's scores
                    t_y = work.tile([128, 4, D], BF16, tag='y', name='t_y')
                    for qq in range(4):
                        pend_y.append((j, qq, t_y))

            p_ot_by_slot = {}
            y_skip = 2
            for idx in range(n_groups):
                si, g = seq[idx]
                while fi < len(fillers) and fillers[fi][0] <= idx:
                    fillers[fi][1]()
                    fi += 1
                if g == 0:
                    p_ot_by_slot[si] = po.tile([128, 512], F32, tag='po',
                                               name='p_ot')
                p_ot_cur = p_ot_by_slot[si]
                t_et, last_mm = emit_scores(si, g)
                if g == GPS - 1:
                    # next j for this head reuses kp rows 112:128. Tile does
                    # NOT track the DMA-write-after-PE-read hazard here (the
                    # reload only gets DMA-queue-ring waits), so add the WAR
                    # edge on this slot's last score matmul explicitly: the
                    # PE stream is in-order, so that one edge covers all 12
                    # score reads of the old rows.
                    j, hh = SLOTS[si]
                    if j + 1 < QC:
                        # issue from the otherwise-idle gpsimd engine: the
                        # WAR wait must not block SP's DMA stream (it would
                        # hold up y-output DMAs and cross-couple queues)
                        dma = nc.gpsimd.dma_start(t_kp[hh][112:128, :],
                                                  apack[hh, j + 1])
                        deps = InstructionNameOrderedSet()
                        deps.add(last_mm.ins.name)
                        dma.ins.add_sync_dependencies_from(deps)
                if pend is not None:
                    emit_av(*pend)
                    if pend[1] == GPS - 1:
                        finish_slot(pend[0], pend[2])
                        if pend_y:
                            # let the divide (recip+mul) clear the DVE chain
                            # before the first y-projection reads t_ot, and
                            # spread the blocks so scores aren't walled off
                            y_skip = 2
                if pend_y:
                    if y_skip > 0:
                        y_skip -= 1
                    elif idx % 2 == 0:
                        emit_y_qq(*pend_y.pop(0))
                pend = (si, g, p_ot_cur, t_et)
            emit_av(*pend)
            # tail: split the last divide into column halves so the first
            # out-projection blocks start as soon as their columns are ready
            lsi, lp_ot = pend[0], pend[2]
            lj, lhh = SLOTS[lsi]
            t_y_last = work.tile([128, 4, D], BF16, tag='y', name='t_y')
            for half in range(2):
                cs0, cs1 = half * 256, half * 256 + 256
                t_rec = work.tile([64, 256], F32, tag='rec', name='t_rec')
                nc.vector.reciprocal(t_rec[:], lp_ot[64:128, cs0:cs1])
                nc.vector.tensor_mul(
                    t_ot[lhh // 2][(lhh % 2) * 64:(lhh % 2) * 64 + 64,
                                   lj * 512 + cs0:lj * 512 + cs1],
                    lp_ot[0:64, cs0:cs1], t_rec[:])
                for qq in (2 * half, 2 * half + 1):
                    emit_y_qq(lj, qq, t_y_last)
            while pend_y:
                emit_y_qq(*pend_y.pop(0))

    nc._state.remove_inst_callback(_chain_cb)
    _split_waits(nc)
    return nc


_NC_CACHE = {}


def _get_nc(with_bias=False):
    if with_bias not in _NC_CACHE:
        _NC_CACHE[with_bias] = _build(with_bias)
    return _NC_CACHE[with_bias]


def _pad_obs(a):
    # lay each head's 16 obs channels at a 32-column boundary (PSUM reads
    # must start at 32-partition-aligned offsets)
    out = np.zeros((a.shape[0], 128), np.float32)
    for hh in range(HPC):
        out[:, hh * 32:hh * 32 + OD] = a[:, hh * OD:(hh + 1) * OD]
    return out


def _host_prep(h, observation_state, Wq, bq, Wk, bk, Wv, bv, Wo, bo,
               Woq, boq, Wok, bok, variable_bias, relative_time_bias,
               with_bias=False):
    f32 = np.float32
    h = np.asarray(h, f32)
    obs = np.asarray(observation_state, f32)
    Kidx = np.arange(N)
    tK = Kidx // V                                     # time bin of each token
    kstat = (Kidx[None, :] % V == np.arange(V)[:, None]).astype(f32)
    bq16 = ((Kidx[None, :] // V) % 16 == np.arange(16)[:, None]).astype(f32)

    in_maps = []
    for c in range(NCORES):
        b, hg = divmod(c, 2)
        h0 = hg * HPC
        cs, ce = h0 * HD, (h0 + HPC) * HD
        os_, oe = h0 * OD, (h0 + HPC) * OD
        qstat = np.empty((HPC, 48, N), f32)
        ap = np.empty((HPC, QC, 16, N), f32)
        for hh in range(HPC):
            head = h0 + hh
            vb = np.asarray(variable_bias[head], f32)
            rtb = np.asarray(relative_time_bias[head], f32)
            qstat[hh, :V] = vb[Kidx % V, :].T          # VB_h[Q%32, j]
            qstat[hh, V:] = bq16
            for j in range(QC):
                # A_hj[s, K] = rtb[16j + s - K//32 + 47]
                idx = 16 * j + np.arange(16)[:, None] - tK[None, :] + (T - 1)
                ap[hh, j] = rtb[idx]
        pdt = ml_dtypes.bfloat16 if PROJ_BF16 else f32
        m = {
            'fhT': np.ascontiguousarray(h[b].reshape(N, D).T).astype(pdt),
            'foT': np.ascontiguousarray(obs[b].reshape(N, 2).T),
            # content / obs scales folded into the q-side weights
            'wq': np.ascontiguousarray(
                np.asarray(Wq, f32)[:, cs:ce] * SCALE).astype(pdt),
            'wk': np.ascontiguousarray(
                np.asarray(Wk, f32)[:, cs:ce]).astype(pdt),
            'wv': np.ascontiguousarray(
                np.asarray(Wv, f32)[:, cs:ce]).astype(pdt),
            'woq': _pad_obs(np.asarray(Woq, f32)[:, os_:oe] * OBS_SCALE),
            'wok': _pad_obs(np.asarray(Wok, f32)[:, os_:oe]),
            'wo': np.ascontiguousarray(
                np.asarray(Wo, f32)[cs:ce, :].reshape(2, 128, D)),
            'vones': np.ones((1, 256), f32),
            'kstat': kstat,
            'qstat': qstat,
            'apack': ap,
        }
        if with_bias:
            m.update({
                'bqr': np.ascontiguousarray(
                    np.asarray(bq, f32)[None, cs:ce] * SCALE),
                'bkr': np.ascontiguousarray(np.asarray(bk, f32)[None, cs:ce]),
                'bvr': np.ascontiguousarray(np.asarray(bv, f32)[None, cs:ce]),
                'boqr': _pad_obs(np.asarray(boq, f32)[None, os_:oe] * OBS_SCALE),
                'bokr': _pad_obs(np.asarray(bok, f32)[None, os_:oe]),
                'onesd': np.ones((1, 512), f32),
            })
        in_maps.append(m)
    return in_maps


def kernel(**inputs):
    with_bias = any(
        np.any(np.asarray(inputs[k])) for k in ('bq', 'bk', 'bv', 'boq', 'bok'))
    nc = _get_nc(with_bias)
    in_maps = _host_prep(**inputs, with_bias=with_bias)
    res = run_bass_kernel_spmd(nc, in_maps, core_ids=list(range(NCORES)))
    bo = np.asarray(inputs['bo'], np.float32)
    outf = np.zeros((B, N, D), np.float32)
    for c in range(NCORES):
        outf[c // 2] += res.results[c]['out'].astype(np.float32)
    outf += bo[None, None, :]
    return outf.reshape(B, T, V, D)
